# revision 1
# baseline (speedup 1.0000x reference)
"""Trainium2 Bass kernel for AudioPreprocessingLayer.

Computes: floor(log2(mel_fb @ (rfft(x*hamming, norm=forward).real ** 2)))
for x of shape (4096, 32, 512), sharded batch-wise across 8 NeuronCores.

Key ideas:
  - rfft(.).real is a matmul with the cosine matrix C[n,k] = cos(2*pi*k*n/512)/512.
    The hamming window folds into it host-side: W = diag(hw) @ C.
  - Mel filterbank column 0 (DC bin) is structurally zero, so only bins 1..256
    are computed -> 256 = 2x128 clean chunks (checked at runtime, with a
    257-bin fallback).
  - x is converted once to bf16; the on-chip transpose runs as REGULAR bf16
    matmuls against an identity (1 cycle/row AND counts as PE activity, so
    the HAM clock gate stays at 2.4 GHz — transpose-mode matmuls don't).
  - floor(log2(m)) for positive fp32 m is exactly
    max(bitcast_int32(m) >> 23, 75) - 127   (the max() also maps the
    mels==0 -> eps=2^-52 case to -52 exactly).
  - Rows are mapped to partitions in blocks of JT=16 per 2048-row macro-group
    (row = 16p + j), so every DMA descriptor covers 16 consecutive DRAM rows
    (32 KB in, 1280 B out).
"""

import os
import sys

for _p in ("/opt/trn_rl_repo",):
    if _p not in sys.path and os.path.isdir(_p):
        sys.path.append(_p)

import numpy as np
import ml_dtypes

import concourse.bass as bass
from concourse import bacc, mybir
from concourse.tile import TileContext
from concourse.bass_utils import run_bass_kernel_spmd
from concourse.masks import make_identity

N_CORES = 8
B, T, FRAME = 4096, 32, 512
R_PER_CORE = (B // N_CORES) * T  # 16384 rows of length 512 per core
N_MELS = 20

f32 = mybir.dt.float32
f32r = mybir.dt.float32r
bf16 = mybir.dt.bfloat16
i32 = mybir.dt.int32


def _ceil_div(a, b):
    return (a + b - 1) // b


def build_graph(R=R_PER_CORE, NF=256, group_r=512, w_dtype=f32r):
    """Build the SPMD Bass graph for one core's shard.

    x:   [R, 512]  f32   rows to transform
    w:   [4, 128, NF] f32  cosine*window matrix, chunked along n
    fbt: [NFC, 128, N_MELS] bf16  mel filterbank transposed+chunked along freq
    out: [R, N_MELS] f32
    """
    assert R % group_r == 0 and group_r % 128 == 0
    RT = group_r // 128          # row subtiles per group (block size k)
    NG = R // group_r            # number of groups
    NQ = FRAME // 128            # 4 n-chunks
    NFC = _ceil_div(NF, 128)     # freq chunks
    f_sizes = [min(128, NF - 128 * c) for c in range(NFC)]

    nc = bacc.Bacc(None, target_bir_lowering=False)
    x_d = nc.declare_dram_parameter("x", [R, FRAME], f32, isOutput=False)
    w_d = nc.declare_dram_parameter("w", [NQ, 128, NF], f32, isOutput=False)
    fbt_d = nc.declare_dram_parameter("fbt", [NFC, 128, N_MELS], bf16, isOutput=False)
    out_d = nc.declare_dram_parameter("out", [R, N_MELS], f32, isOutput=True)

    with TileContext(nc) as tc:
        with (
            tc.tile_pool(name="consts", bufs=1) as consts,
            tc.tile_pool(name="xb", bufs=4) as xb_pool,
            tc.tile_pool(name="xq", bufs=2) as xq_pool,
            tc.tile_pool(name="mag", bufs=2) as mag_pool,
            tc.tile_pool(name="fin", bufs=3) as fin_pool,
            tc.tile_pool(
                name="ps_xt", bufs=(4 if group_r <= 256 else 3), space="PSUM"
            ) as ps_xt_pool,
            tc.tile_pool(
                name="ps_y", bufs=(2 if NFC <= 2 else 1), space="PSUM"
            ) as ps_y_pool,
            tc.tile_pool(
                name="ps_m", bufs=(2 if group_r <= 256 else 1), space="PSUM"
            ) as ps_m_pool,
        ):
            # ---- constants ----
            ident = consts.tile([128, 128], bf16)
            make_identity(nc, ident)

            w_sb = consts.tile([128, NQ, NF], f32)
            nc.sync.dma_start(out=w_sb, in_=w_d.rearrange("q p f -> p q f"))
            # fp32r operands must be produced pre-rounded; one-time copy
            w_r = consts.tile([128, NQ, NF], w_dtype)
            nc.vector.tensor_copy(w_r, w_sb)

            fbt_sb = consts.tile([128, NFC, N_MELS], bf16)
            nc.sync.dma_start(out=fbt_sb, in_=fbt_d.rearrange("c p m -> p c m"))

            # compute groups per DMA macro-group; first ones small so the
            # pipeline fills quickly
            n_groups = R // group_r
            gpm = 2048 // group_r
            if n_groups >= 2 * gpm:
                gpm_list = [1, gpm - 1] + [gpm] * ((n_groups - gpm) // gpm)
            else:
                gpm_list = [1] * n_groups
            assert sum(gpm_list) == n_groups, (gpm_list, n_groups)

            # flat per-group schedule over variable-size macro-groups
            groups = []   # (macro, gg) per group
            macros = []   # per macro: dict(m0, GPM, JT)
            m0 = 0
            for mg, GPM in enumerate(gpm_list):
                macros.append({"m0": m0, "GPM": GPM, "JT": GPM * RT})
                for gg in range(GPM):
                    groups.append((mg, gg))
                m0 += GPM * group_r

            st = {}  # per-group transpose-stage outputs

            def stage_T(g):
                mg, gg = groups[g]
                mac = macros[mg]
                if gg == 0:
                    # load macro as a CASTING DMA (f32 dram -> bf16 sbuf);
                    # row m0 + JT*p + j -> partition p, slot j (up to 32 KB
                    # contiguous DRAM per partition = big descriptors)
                    JT = mac["JT"]
                    xb_sb = xb_pool.tile([128, JT, FRAME], bf16, name="xb_sb")
                    nc.gpsimd.dma_start(
                        out=xb_sb,
                        in_=x_d[
                            mac["m0"] : mac["m0"] + JT * 128, :
                        ].rearrange("(p j) n -> p j n", j=JT),
                    )
                    mac["xb"] = xb_sb
                    mac["e_sb"] = fin_pool.tile(
                        [128, JT * N_MELS], i32, tag="e_sb", name="e_sb"
                    )
                xb_sb = mac["xb"]
                # transpose via REGULAR bf16 matmuls (counts for HAM);
                # one single-bank PSUM slot per n-chunk
                xq_sb = []
                for q in range(NQ):
                    t = ps_xt_pool.tile(
                        [128, group_r], f32, name=f"xt{q}", tag="xt"
                    )
                    for j in range(RT):
                        nc.tensor.matmul(
                            t[:, j * 128 : (j + 1) * 128],
                            xb_sb[:, gg * RT + j, q * 128 : (q + 1) * 128],
                            ident,
                            start=True,
                            stop=True,
                        )
                    # copy PSUM -> SBUF as f32r (exact for bf16-valued x,
                    # keeps matmul 1 all-32-bit with full-precision W)
                    dst = xq_pool.tile(
                        [128, group_r], f32r, name=f"xq{q}", tag=f"xq{q}"
                    )
                    xq_sb.append(dst)
                    if q % 2 == 0:
                        nc.vector.tensor_copy(dst, t)
                    else:
                        nc.scalar.copy(dst, t)
                st[g] = xq_sb

            def stage_M1(g):
                # matmul 1: yT[f, r] += W[n, f].T @ xT[n, r]; then square
                xq_sb = st.pop(g)
                y_ps = ps_y_pool.tile([128, NFC, group_r], f32, name="y_ps")
                for c in range(NFC):
                    fs = f_sizes[c]
                    for q in range(NQ):
                        nc.tensor.matmul(
                            y_ps[:fs, c, :],
                            w_r[:, q, 128 * c : 128 * c + fs],
                            xq_sb[q],
                            start=(q == 0),
                            stop=(q == NQ - 1),
                        )
                # square: magT = yT*yT (fused, psum -> sbuf bf16)
                mag_sb = mag_pool.tile([128, NFC, group_r], bf16, name="mag_sb")
                nc.scalar.activation(
                    mag_sb, y_ps, mybir.ActivationFunctionType.Square
                )
                st[("mag", g)] = mag_sb

            def stage_M2(g):
                mg, gg = groups[g]
                mac = macros[mg]
                mag_sb = st.pop(("mag", g))
                # matmul 2: mels[r, m] += magT[f, r].T @ fbt[f, m]
                mels_ps = ps_m_pool.tile([128, RT * N_MELS], f32, name="mels_ps")
                for j in range(RT):
                    for c in range(NFC):
                        fs = f_sizes[c]
                        nc.tensor.matmul(
                            mels_ps[:, j * N_MELS : (j + 1) * N_MELS],
                            mag_sb[:fs, c, j * 128 : (j + 1) * 128],
                            fbt_sb[:fs, c, :],
                            start=(c == 0),
                            stop=(c == NFC - 1),
                        )
                # exponent bits out of PSUM (rest of finalize is batched)
                nc.vector.tensor_scalar(
                    mac["e_sb"][:, gg * RT * N_MELS : (gg + 1) * RT * N_MELS],
                    mels_ps.bitcast(i32),
                    23,
                    None,
                    mybir.AluOpType.logical_shift_right,
                )
                if gg == mac["GPM"] - 1:
                    # finalize: floor(log2(m)) = max(bits >> 23, 75) - 127
                    JT = mac["JT"]
                    e_sb = mac["e_sb"]
                    ef_sb = fin_pool.tile([128, JT * N_MELS], f32, tag="ef_sb", name="ef_sb")
                    nc.vector.tensor_copy(ef_sb, e_sb)
                    o_sb = fin_pool.tile([128, JT * N_MELS], f32, tag="o_sb", name="o_sb")
                    nc.vector.tensor_scalar(
                        o_sb,
                        ef_sb,
                        75.0,
                        127.0,
                        mybir.AluOpType.max,
                        mybir.AluOpType.subtract,
                    )
                    # store: one DMA per macro, JT rows per partition
                    nc.sync.dma_start(
                        out=out_d[
                            mac["m0"] : mac["m0"] + JT * 128, :
                        ].rearrange("(p j) m -> p (j m)", j=JT),
                        in_=o_sb,
                    )

            for g in range(len(groups)):
                stage_T(g)
                stage_M1(g)
                stage_M2(g)
    nc.compile()
    return nc


def _prep_weights(filter_banks, hw):
    """Host-side: cosine*window matrix and chunked transposed filterbank."""
    fb = np.asarray(filter_banks, dtype=np.float32)
    n_mels, n_bins = fb.shape  # (20, 257)
    assert n_mels == N_MELS and n_bins == FRAME // 2 + 1

    if np.all(fb[:, 0] == 0.0):
        k0 = 1  # DC bin unused by the filterbank (structurally true)
    else:
        k0 = 0
    NF = n_bins - k0

    n = np.arange(FRAME, dtype=np.float64)
    k = np.arange(k0, n_bins, dtype=np.float64)
    C = np.cos(2.0 * np.pi * np.outer(n, k) / FRAME) / FRAME
    W = (np.asarray(hw, dtype=np.float64)[:, None] * C).astype(np.float32)
    NQ = FRAME // 128
    w_chunks = np.ascontiguousarray(W.reshape(NQ, 128, NF))

    NFC = _ceil_div(NF, 128)
    fbt = np.zeros((NFC, 128, N_MELS), dtype=ml_dtypes.bfloat16)
    fbT = fb[:, k0:].T.astype(ml_dtypes.bfloat16)  # [NF, 20]
    for c in range(NFC):
        fs = min(128, NF - 128 * c)
        fbt[c, :fs, :] = fbT[128 * c : 128 * c + fs, :]
    return w_chunks, fbt, NF


_CACHE = {}


def _get_graph(R, NF, group_r):
    key = (R, NF, group_r)
    if key not in _CACHE:
        _CACHE[key] = build_graph(R, NF, group_r)
    return _CACHE[key]


def kernel(inputs, filter_banks, hw, _trace=False, _group_r=512):
    x = np.ascontiguousarray(np.asarray(inputs, dtype=np.float32))
    assert x.shape == (B, T, FRAME), x.shape
    w_chunks, fbt, NF = _prep_weights(filter_banks, hw)

    shards = x.reshape(N_CORES, B // N_CORES * T, FRAME)
    nc = _get_graph(R_PER_CORE, NF, _group_r)
    in_maps = [
        {"x": shards[i], "w": w_chunks, "fbt": fbt} for i in range(N_CORES)
    ]
    res = run_bass_kernel_spmd(
        nc, in_maps, core_ids=list(range(N_CORES)), trace=_trace
    )
    out = np.stack([res.results[i]["out"] for i in range(N_CORES)], axis=0)
    out = out.reshape(B, T, N_MELS, 1).astype(np.float32)
    if _trace:
        kernel._last_result = res
    return out



# revision 4
# speedup vs baseline: 1.0613x; 1.0613x over previous
"""Trainium2 Bass kernel for AudioPreprocessingLayer.

Computes: floor(log2(mel_fb @ (rfft(x*hamming, norm=forward).real ** 2)))
for x of shape (4096, 32, 512), sharded batch-wise across 8 NeuronCores.

Key ideas:
  - rfft(.).real is a matmul with the cosine matrix C[n,k] = cos(2*pi*k*n/512)/512.
  - Parity fold: C[n+256, k] = (-1)^k C[n, k], so with
      ue[n] = hx[n] + hx[n+256],  uo[n] = hx[n] - hx[n+256]   (hx = hw*x)
    the even-k bins need only ue (256-long contraction) and the odd-k bins
    only uo — the DFT matmul work halves vs. the unfolded 512-contraction.
  - The host hands the kernel x already TRANSPOSED to [n, r] layout (a pure
    permutation, done during sharding), so no on-chip transpose is needed:
    the DMA-loaded tiles feed the DFT matmul directly with n on partitions.
  - The row order within each DMA macro-block is permuted host-side so the
    OUTPUT rows land partition-contiguous (big store descriptors), i.e. the
    input permutation absorbs both the transpose and the store layout.
  - fp16 end-to-end for x/hx/u and the cosine weights (better precision than
    the bf16 baseline and full PE speed); mag/filterbank in bf16 (fp16 would
    flush y^2 subnormals); PSUM accumulation in f32.
  - floor(log2(m)) for positive fp32 m is exactly
    max(bitcast_int32(m) >> 23, 75) - 127   (the max() also maps the
    mels==0 -> eps=2^-52 case to -52 exactly).
"""

import os
import sys

for _p in ("/opt/trn_rl_repo",):
    if _p not in sys.path and os.path.isdir(_p):
        sys.path.append(_p)

import numpy as np
import ml_dtypes

import concourse.bass as bass
from concourse import bacc, mybir
from concourse.tile import TileContext
from concourse.bass_utils import run_bass_kernel_spmd

N_CORES = 8
B, T, FRAME = 4096, 32, 512
R = (B // N_CORES) * T  # 16384 rows of length 512 per core
N_MELS = 20
NQ = FRAME // 128  # 4 n-chunks of the transposed input
GR = 512  # rows per compute group (one PSUM bank of f32)

# DMA macro-blocks (rows): first ones small so the pipeline fills quickly.
MACROS = [(0, 512), (512, 1536)] + [(2048 + 2048 * i, 2048) for i in range(7)]
assert sum(rb for _, rb in MACROS) == R

f32 = mybir.dt.float32
f16 = mybir.dt.float16
bf16 = mybir.dt.bfloat16
i32 = mybir.dt.int32


def build_graph():
    """SPMD Bass graph for one core's shard.

    xt:  [NQ, 128, R] f16   transposed rows: xt[c, p, r] = x[perm(r), 128c+p]
    ce:  [2, 128, 128] f16  cos matrix for even k (k=2,4,...,256), n-chunked
    co:  [2, 128, 128] f16  cos matrix for odd k (k=1,3,...,255), n-chunked
    fbt: [2, 128, N_MELS] bf16  mel filterbank, split by k parity
    hwc: [128, NQ] f32      hamming window, n-chunked per partition
    out: [R, N_MELS] f32
    """
    nc = bacc.Bacc(None, target_bir_lowering=False)
    xt_d = nc.declare_dram_parameter("xt", [NQ, 128, R], f16, isOutput=False)
    ce_d = nc.declare_dram_parameter("ce", [2, 128, 128], f16, isOutput=False)
    co_d = nc.declare_dram_parameter("co", [2, 128, 128], f16, isOutput=False)
    fbt_d = nc.declare_dram_parameter("fbt", [2, 128, N_MELS], bf16, isOutput=False)
    hw_d = nc.declare_dram_parameter("hwc", [128, NQ], f32, isOutput=False)
    out_d = nc.declare_dram_parameter("out", [R, N_MELS], f32, isOutput=True)

    with TileContext(nc) as tc:
        with (
            tc.tile_pool(name="consts", bufs=1) as consts,
            tc.tile_pool(name="xt", bufs=2) as xt_pool,
            tc.tile_pool(name="hx", bufs=2) as hx_pool,
            tc.tile_pool(name="u", bufs=2) as u_pool,
            tc.tile_pool(name="mag", bufs=3) as mag_pool,
            tc.tile_pool(name="fin", bufs=2) as fin_pool,
            tc.tile_pool(name="ps_y", bufs=2, space="PSUM") as ps_y_pool,
            tc.tile_pool(name="ps_m", bufs=2, space="PSUM") as ps_m_pool,
        ):
            ce_sb = consts.tile([128, 2, 128], f16)
            nc.sync.dma_start(out=ce_sb, in_=ce_d.rearrange("c p k -> p c k"))
            co_sb = consts.tile([128, 2, 128], f16)
            nc.sync.dma_start(out=co_sb, in_=co_d.rearrange("c p k -> p c k"))
            fbt_sb = consts.tile([128, 2, N_MELS], bf16)
            nc.sync.dma_start(out=fbt_sb, in_=fbt_d.rearrange("e j m -> j e m"))
            hw_sb = consts.tile([128, NQ], f32)
            nc.sync.dma_start(out=hw_sb, in_=hw_d[:, :])

            for r0, RB in MACROS:
                S = RB // 128  # output slots per macro
                xt_sb = xt_pool.tile([128, NQ, RB], f16, name="xt_sb")
                nc.gpsimd.dma_start(
                    out=xt_sb,
                    in_=xt_d[:, :, r0 : r0 + RB].rearrange("c p r -> p c r"),
                )
                # window: hx[c] = x[c] * hw[c]  (per-partition scalar)
                hx_sb = hx_pool.tile([128, NQ, RB], f16, name="hx_sb")
                for c in range(NQ):
                    nc.vector.tensor_scalar(
                        hx_sb[:, c],
                        xt_sb[:, c],
                        hw_sb[:, c : c + 1],
                        None,
                        mybir.AluOpType.mult,
                    )
                # parity fold: u = [ue0, ue1, uo0, uo1]
                u_sb = u_pool.tile([128, NQ, RB], f16, name="u_sb")
                for c in range(2):
                    nc.vector.tensor_add(u_sb[:, c], hx_sb[:, c], hx_sb[:, c + 2])
                    nc.vector.tensor_sub(u_sb[:, 2 + c], hx_sb[:, c], hx_sb[:, c + 2])

                e_sb = fin_pool.tile([128, S * N_MELS], i32, tag="e_sb", name="e_sb")
                for g in range(RB // GR):
                    r = slice(g * GR, (g + 1) * GR)
                    # DFT: y[k, r] for even/odd k (f32 PSUM accumulate)
                    y_ps = ps_y_pool.tile([128, 2, GR], f32, name="y_ps")
                    for c in range(2):
                        nc.tensor.matmul(
                            y_ps[:, 0, :], ce_sb[:, c, :], u_sb[:, c, r],
                            start=(c == 0), stop=(c == 1),
                        )
                    for c in range(2):
                        nc.tensor.matmul(
                            y_ps[:, 1, :], co_sb[:, c, :], u_sb[:, 2 + c, r],
                            start=(c == 0), stop=(c == 1),
                        )
                    # mag = y^2 (fused PSUM -> SBUF bf16)
                    mag_sb = mag_pool.tile([128, 2, GR], bf16, name="mag_sb")
                    nc.scalar.activation(
                        mag_sb, y_ps, mybir.ActivationFunctionType.Square
                    )
                    # mel: mels[r, m] += mag[k, r].T @ fbt[k, m]
                    mels_ps = ps_m_pool.tile([128, (GR // 128) * N_MELS], f32,
                                             name="mels_ps")
                    for j in range(GR // 128):
                        jj = slice(j * 128, (j + 1) * 128)
                        for e in range(2):
                            nc.tensor.matmul(
                                mels_ps[:, j * N_MELS : (j + 1) * N_MELS],
                                mag_sb[:, e, jj], fbt_sb[:, e, :],
                                start=(e == 0), stop=(e == 1),
                            )
                    # exponent bits out of PSUM (GPSIMD can't read PSUM)
                    nc.vector.tensor_scalar(
                        e_sb[:, g * 4 * N_MELS : (g + 1) * 4 * N_MELS],
                        mels_ps.bitcast(i32),
                        23,
                        None,
                        mybir.AluOpType.logical_shift_right,
                    )
                # finalize: floor(log2(m)) = max(bits >> 23, 75) - 127
                ef_sb = fin_pool.tile([128, S * N_MELS], f32, tag="ef_sb", name="ef_sb")
                nc.gpsimd.tensor_copy(ef_sb, e_sb)
                o_sb = fin_pool.tile([128, S * N_MELS], f32, tag="o_sb", name="o_sb")
                nc.gpsimd.tensor_scalar(
                    o_sb, ef_sb, 75.0, 127.0,
                    mybir.AluOpType.max, mybir.AluOpType.subtract,
                )
                # store: rows r0 + p*S + s are partition-contiguous in DRAM
                nc.sync.dma_start(
                    out=out_d[r0 : r0 + RB, :].rearrange("(p j) m -> p (j m)", j=S),
                    in_=o_sb,
                )
    nc.compile()
    return nc


def _prep_weights(filter_banks, hw):
    fb = np.asarray(filter_banks, dtype=np.float32)
    n_mels, n_bins = fb.shape  # (20, 257)
    assert n_mels == N_MELS and n_bins == FRAME // 2 + 1
    assert np.all(fb[:, 0] == 0.0), "parity-fold kernel needs an unused DC bin"

    k_even = np.arange(2, 257, 2)  # 128 bins: 2..256
    k_odd = np.arange(1, 256, 2)  # 128 bins: 1..255
    n = np.arange(256, dtype=np.float64)
    ce = (np.cos(2.0 * np.pi * np.outer(n, k_even) / FRAME) / FRAME)
    co = (np.cos(2.0 * np.pi * np.outer(n, k_odd) / FRAME) / FRAME)
    ce = ce.reshape(2, 128, 128).astype(np.float16)
    co = co.reshape(2, 128, 128).astype(np.float16)

    fbt = np.empty((2, 128, N_MELS), dtype=ml_dtypes.bfloat16)
    fbt[0] = fb[:, k_even].T
    fbt[1] = fb[:, k_odd].T

    hwc = np.ascontiguousarray(
        np.asarray(hw, dtype=np.float32).reshape(NQ, 128).T
    )  # [128, NQ]
    return ce, co, fbt, hwc


def _prep_inputs(x):
    """Shard, permute, transpose, cast: per core xt[c, p, r] with the macro-
    local row order r = 128*s + p_out chosen so stores are contiguous."""
    x16 = x.reshape(N_CORES, R, FRAME).astype(np.float16)
    parts = []
    for r0, RB in MACROS:
        S = RB // 128
        blk = x16[:, r0 : r0 + RB, :].reshape(N_CORES, 128, S, FRAME)
        # [core, p, s, n] -> [core, n, s, p] -> [core, NQ, 128, S*128]
        t = blk.transpose(0, 3, 2, 1).reshape(N_CORES, NQ, 128, RB)
        parts.append(t)
    xt = np.concatenate(parts, axis=3)  # [core, NQ, 128, R]
    return np.ascontiguousarray(xt)


_CACHE = {}


def _get_graph():
    if "nc" not in _CACHE:
        _CACHE["nc"] = build_graph()
    return _CACHE["nc"]


def kernel(inputs, filter_banks, hw, _trace=False):
    x = np.ascontiguousarray(np.asarray(inputs, dtype=np.float32))
    assert x.shape == (B, T, FRAME), x.shape
    ce, co, fbt, hwc = _prep_weights(filter_banks, hw)
    xt = _prep_inputs(x)

    nc = _get_graph()
    in_maps = [
        {"xt": xt[i], "ce": ce, "co": co, "fbt": fbt, "hwc": hwc}
        for i in range(N_CORES)
    ]
    res = run_bass_kernel_spmd(
        nc, in_maps, core_ids=list(range(N_CORES)), trace=_trace
    )
    out = np.stack([res.results[i]["out"] for i in range(N_CORES)], axis=0)
    out = out.reshape(B, T, N_MELS, 1).astype(np.float32)
    if _trace:
        kernel._last_result = res
    return out


# revision 7
# speedup vs baseline: 1.1630x; 1.0958x over previous
"""Trainium2 Bass kernel for AudioPreprocessingLayer.

Computes: floor(log2(mel_fb @ (rfft(x*hamming, norm=forward).real ** 2)))
for x of shape (4096, 32, 512), sharded batch-wise across 8 NeuronCores.

Key ideas:
  - rfft(.).real is a matmul with the cosine matrix C[n,k] = cos(2*pi*k*n/512)/512.
  - Parity fold: C[n+256, k] = (-1)^k C[n, k], so the even-k bins need only
    ue[n] = hw[n]x[n] + hw[n+256]x[n+256] and the odd-k bins only
    uo[n] = hw[n]x[n] - hw[n+256]x[n+256] — a 256-long contraction instead
    of 512: the DFT matmul work halves.
  - Window-in-weights: ue = hw_lo * (x_lo + g*x_hi) with g = hw_hi/hw_lo,
    and the outer hw_lo folds into the cosine weights. So the whole
    window+fold is 4 scalar_tensor_tensor ops per macro-block
    (out = (x_hi * ±g) + x_lo, g a per-partition scalar).
  - The host hands the kernel x already TRANSPOSED to [n, r] layout (a pure
    permutation, done during sharding), so no on-chip transpose is needed:
    the DMA-loaded tiles feed the DFT matmul directly with n on partitions.
  - The row order within each DMA macro-block is permuted host-side so the
    OUTPUT rows land partition-contiguous (big store descriptors).
  - fp16 end-to-end for x/u and the windowed cosine weights (better
    precision than a bf16 pipeline and full PE speed); mag/filterbank in
    bf16 (fp16 would flush y^2 subnormals); PSUM accumulation in f32.
  - floor(log2(m)) for positive fp32 m is exactly
    max(bitcast_int32(m) >> 23, 75) - 127   (the max() also maps the
    mels==0 -> eps=2^-52 case to -52 exactly).
"""

import os
import sys

for _p in ("/opt/trn_rl_repo",):
    if _p not in sys.path and os.path.isdir(_p):
        sys.path.append(_p)

import numpy as np
import ml_dtypes

import concourse.bass as bass
from concourse import bacc, mybir
from concourse.tile import TileContext
from concourse.bass_utils import run_bass_kernel_spmd

N_CORES = 8
B, T, FRAME = 4096, 32, 512
R = (B // N_CORES) * T  # 16384 rows of length 512 per core
N_MELS = 20
NQ = FRAME // 128  # 4 n-chunks of the transposed input
GR = 512  # rows per compute group (one PSUM bank of f32)

# DMA macro-blocks (rows): first ones small so the pipeline fills quickly.
MACROS = [(0, 512), (512, 1536)] + [(2048 + 2048 * i, 2048) for i in range(7)]
assert sum(rb for _, rb in MACROS) == R

f32 = mybir.dt.float32
f16 = mybir.dt.float16
bf16 = mybir.dt.bfloat16
i32 = mybir.dt.int32


def build_graph():
    """SPMD Bass graph for one core's shard.

    xt:  [NQ, 128, R] f16   transposed rows: xt[c, p, r] = x[perm(r), 128c+p]
    ce:  [2, 128, 128] f16  diag(hw_lo) @ cos matrix, even k (2,4,...,256)
    co:  [2, 128, 128] f16  diag(hw_lo) @ cos matrix, odd k (1,3,...,255)
    fbt: [2, 128, N_MELS] bf16  mel filterbank, split by k parity
    gr:  [128, 4] f32       window ratio hw_hi/hw_lo: cols [+g0,+g1,-g0,-g1]
    out: [R, N_MELS] f32
    """
    nc = bacc.Bacc(None, target_bir_lowering=False)
    xt_d = nc.declare_dram_parameter("xt", [NQ, 128, R], f16, isOutput=False)
    ce_d = nc.declare_dram_parameter("ce", [2, 128, 128], f16, isOutput=False)
    co_d = nc.declare_dram_parameter("co", [2, 128, 128], f16, isOutput=False)
    fbt_d = nc.declare_dram_parameter("fbt", [2, 128, N_MELS], bf16, isOutput=False)
    g_d = nc.declare_dram_parameter("gr", [128, 4], f32, isOutput=False)
    out_d = nc.declare_dram_parameter("out", [R, N_MELS], f32, isOutput=True)

    with TileContext(nc) as tc:
        with (
            tc.tile_pool(name="consts", bufs=1) as consts,
            tc.tile_pool(name="xt", bufs=2) as xt_pool,
            tc.tile_pool(name="u", bufs=2) as u_pool,
            tc.tile_pool(name="mag", bufs=3) as mag_pool,
            tc.tile_pool(name="fin", bufs=2) as fin_pool,
            tc.tile_pool(name="ps_y", bufs=2, space="PSUM") as ps_y_pool,
            tc.tile_pool(name="ps_m", bufs=2, space="PSUM") as ps_m_pool,
        ):
            ce_sb = consts.tile([128, 2, 128], f16)
            nc.sync.dma_start(out=ce_sb, in_=ce_d.rearrange("c p k -> p c k"))
            co_sb = consts.tile([128, 2, 128], f16)
            nc.sync.dma_start(out=co_sb, in_=co_d.rearrange("c p k -> p c k"))
            fbt_sb = consts.tile([128, 2, N_MELS], bf16)
            nc.sync.dma_start(out=fbt_sb, in_=fbt_d.rearrange("e j m -> j e m"))
            g_sb = consts.tile([128, 4], f32)
            nc.sync.dma_start(out=g_sb, in_=g_d[:, :])

            for r0, RB in MACROS:
                S = RB // 128  # output slots per macro
                xt_sb = xt_pool.tile([128, NQ, RB], f16, name="xt_sb")
                nc.gpsimd.dma_start(
                    out=xt_sb,
                    in_=xt_d[:, :, r0 : r0 + RB].rearrange("c p r -> p c r"),
                )
                # fused window+fold: u[c] = x[c] +/- g[c]*x[c+2]
                # (u = [s0, s1, d0, d1]; hw_lo is folded into ce/co)
                u_sb = u_pool.tile([128, NQ, RB], f16, name="u_sb")
                for c in range(2):
                    nc.vector.scalar_tensor_tensor(
                        u_sb[:, c], xt_sb[:, c + 2], g_sb[:, c : c + 1],
                        xt_sb[:, c],
                        mybir.AluOpType.mult, mybir.AluOpType.add,
                    )
                    nc.vector.scalar_tensor_tensor(
                        u_sb[:, 2 + c], xt_sb[:, c + 2], g_sb[:, 2 + c : 3 + c],
                        xt_sb[:, c],
                        mybir.AluOpType.mult, mybir.AluOpType.add,
                    )

                e_sb = fin_pool.tile([128, S * N_MELS], i32, tag="e_sb", name="e_sb")
                for g in range(RB // GR):
                    r = slice(g * GR, (g + 1) * GR)
                    # DFT: y[k, r] for even/odd k (f32 PSUM accumulate)
                    y_ps = ps_y_pool.tile([128, 2, GR], f32, name="y_ps")
                    for c in range(2):
                        nc.tensor.matmul(
                            y_ps[:, 0, :], ce_sb[:, c, :], u_sb[:, c, r],
                            start=(c == 0), stop=(c == 1),
                        )
                    for c in range(2):
                        nc.tensor.matmul(
                            y_ps[:, 1, :], co_sb[:, c, :], u_sb[:, 2 + c, r],
                            start=(c == 0), stop=(c == 1),
                        )
                    # mag = y^2 (fused PSUM -> SBUF bf16)
                    mag_sb = mag_pool.tile([128, 2, GR], bf16, name="mag_sb")
                    nc.scalar.activation(
                        mag_sb, y_ps, mybir.ActivationFunctionType.Square
                    )
                    # mel: mels[r, m] += mag[k, r].T @ fbt[k, m]
                    mels_ps = ps_m_pool.tile([128, (GR // 128) * N_MELS], f32,
                                             name="mels_ps")
                    for j in range(GR // 128):
                        jj = slice(j * 128, (j + 1) * 128)
                        for e in range(2):
                            nc.tensor.matmul(
                                mels_ps[:, j * N_MELS : (j + 1) * N_MELS],
                                mag_sb[:, e, jj], fbt_sb[:, e, :],
                                start=(e == 0), stop=(e == 1),
                            )
                    # exponent bits out of PSUM
                    nc.vector.tensor_scalar(
                        e_sb[:, g * 4 * N_MELS : (g + 1) * 4 * N_MELS],
                        mels_ps.bitcast(i32),
                        23,
                        None,
                        mybir.AluOpType.logical_shift_right,
                    )
                # finalize: floor(log2(m)) = max(bits >> 23, 75) - 127
                ef_sb = fin_pool.tile([128, S * N_MELS], f32, tag="ef_sb", name="ef_sb")
                nc.vector.tensor_copy(ef_sb, e_sb)
                o_sb = fin_pool.tile([128, S * N_MELS], f32, tag="o_sb", name="o_sb")
                nc.vector.tensor_scalar(
                    o_sb, ef_sb, 75.0, 127.0,
                    mybir.AluOpType.max, mybir.AluOpType.subtract,
                )
                # store: rows r0 + p*S + s are partition-contiguous in DRAM
                nc.sync.dma_start(
                    out=out_d[r0 : r0 + RB, :].rearrange("(p j) m -> p (j m)", j=S),
                    in_=o_sb,
                )
    nc.compile()
    return nc


def _prep_weights(filter_banks, hw):
    fb = np.asarray(filter_banks, dtype=np.float32)
    n_mels, n_bins = fb.shape  # (20, 257)
    assert n_mels == N_MELS and n_bins == FRAME // 2 + 1
    assert np.all(fb[:, 0] == 0.0), "parity-fold kernel needs an unused DC bin"

    k_even = np.arange(2, 257, 2)  # 128 bins: 2..256
    k_odd = np.arange(1, 256, 2)  # 128 bins: 1..255
    n = np.arange(256, dtype=np.float64)
    hw64 = np.asarray(hw, dtype=np.float64)
    wlo = hw64[:256, None]
    ce = (wlo * np.cos(2.0 * np.pi * np.outer(n, k_even) / FRAME) / FRAME)
    co = (wlo * np.cos(2.0 * np.pi * np.outer(n, k_odd) / FRAME) / FRAME)
    ce = ce.reshape(2, 128, 128).astype(np.float16)
    co = co.reshape(2, 128, 128).astype(np.float16)

    fbt = np.empty((2, 128, N_MELS), dtype=ml_dtypes.bfloat16)
    fbt[0] = fb[:, k_even].T
    fbt[1] = fb[:, k_odd].T

    g = (hw64[256:] / hw64[:256]).astype(np.float32)  # [256]
    gr = np.empty((128, 4), dtype=np.float32)
    gr[:, 0] = g[:128]
    gr[:, 1] = g[128:]
    gr[:, 2] = -g[:128]
    gr[:, 3] = -g[128:]
    return ce, co, fbt, gr


def _prep_inputs(x):
    """Shard, permute, transpose, cast: per core xt[c, p, r] with the macro-
    local row order r = 128*s + p_out chosen so stores are contiguous."""
    x16 = x.reshape(N_CORES, R, FRAME).astype(np.float16)
    parts = []
    for r0, RB in MACROS:
        S = RB // 128
        blk = x16[:, r0 : r0 + RB, :].reshape(N_CORES, 128, S, FRAME)
        # [core, p, s, n] -> [core, n, s, p] -> [core, NQ, 128, S*128]
        t = blk.transpose(0, 3, 2, 1).reshape(N_CORES, NQ, 128, RB)
        parts.append(t)
    xt = np.concatenate(parts, axis=3)  # [core, NQ, 128, R]
    return np.ascontiguousarray(xt)


_CACHE = {}


def _get_graph():
    if "nc" not in _CACHE:
        _CACHE["nc"] = build_graph()
    return _CACHE["nc"]


def kernel(inputs, filter_banks, hw, _trace=False):
    x = np.ascontiguousarray(np.asarray(inputs, dtype=np.float32))
    assert x.shape == (B, T, FRAME), x.shape
    ce, co, fbt, gr = _prep_weights(filter_banks, hw)
    xt = _prep_inputs(x)

    nc = _get_graph()
    in_maps = [
        {"xt": xt[i], "ce": ce, "co": co, "fbt": fbt, "gr": gr}
        for i in range(N_CORES)
    ]
    res = run_bass_kernel_spmd(
        nc, in_maps, core_ids=list(range(N_CORES)), trace=_trace
    )
    out = np.stack([res.results[i]["out"] for i in range(N_CORES)], axis=0)
    out = out.reshape(B, T, N_MELS, 1).astype(np.float32)
    if _trace:
        kernel._last_result = res
    return out


# revision 14
# speedup vs baseline: 1.2206x; 1.0496x over previous
"""Trainium2 Bass kernel for AudioPreprocessingLayer.

Computes: floor(log2(mel_fb @ (rfft(x*hamming, norm=forward).real ** 2)))
for x of shape (4096, 32, 512), sharded batch-wise across 8 NeuronCores.

Key ideas:
  - rfft(.).real is a matmul with the cosine matrix C[n,k] = cos(2*pi*k*n/512)/512.
  - Parity fold: C[n+256, k] = (-1)^k C[n, k], so the even-k bins need only
    ue[n] = hw[n]x[n] + hw[n+256]x[n+256] and the odd-k bins only
    uo[n] = hw[n]x[n] - hw[n+256]x[n+256] — a 256-long contraction instead
    of 512: the DFT matmul work halves.
  - Window-in-weights: ue = hw_lo * (x_lo + g*x_hi) with g = hw_hi/hw_lo,
    and the outer hw_lo folds into the cosine weights. So the whole
    window+fold is 4 scalar_tensor_tensor ops per macro-block
    (out = (x_hi * ±g) + x_lo, g a per-partition scalar).
  - The host hands the kernel x already TRANSPOSED to [n, r] layout (a pure
    permutation, done during sharding), so no on-chip transpose is needed:
    the DMA-loaded tiles feed the DFT matmul directly with n on partitions.
  - The row order within each DMA macro-block is permuted host-side so the
    OUTPUT rows land partition-contiguous (big store descriptors).
  - fp16 end-to-end for x/u and the windowed cosine weights (better
    precision than a bf16 pipeline and full PE speed); mag/filterbank in
    bf16 (fp16 would flush y^2 subnormals); PSUM accumulation in f32.
  - floor(log2(m)) for positive fp32 m is exactly
    max(bitcast_int32(m) >> 23, 75) - 127   (the max() also maps the
    mels==0 -> eps=2^-52 case to -52 exactly).
"""

import os
import sys

for _p in ("/opt/trn_rl_repo",):
    if _p not in sys.path and os.path.isdir(_p):
        sys.path.append(_p)

import numpy as np
import ml_dtypes

import concourse.bass as bass
from concourse import bacc, mybir
from concourse.tile import TileContext
from concourse.bass_utils import run_bass_kernel_spmd

N_CORES = 8
B, T, FRAME = 4096, 32, 512
R = (B // N_CORES) * T  # 16384 rows of length 512 per core
N_MELS = 20
NQ = FRAME // 128  # 4 n-chunks of the transposed input
GR = 512  # rows per compute group (one PSUM bank of f32)

# DMA macro-blocks (rows): first ones small so the pipeline fills quickly.
MACROS = [(0, 512), (512, 1536)] + [(2048 + 2048 * i, 2048) for i in range(7)]
assert sum(rb for _, rb in MACROS) == R

f32 = mybir.dt.float32
f16 = mybir.dt.float16
bf16 = mybir.dt.bfloat16
i32 = mybir.dt.int32


def build_graph():
    """SPMD Bass graph for one core's shard.

    xt:  [NQ, 128, R] f16   transposed rows: xt[c, p, r] = x[perm(r), 128c+p]
    ce:  [2, 128, 128] f16  diag(hw_lo) @ cos matrix, even k (2,4,...,256)
    wo:  [NQ, 128, 128] f16 full windowed cos matrix, odd k (1,3,...,255)
    fbt: [2, 128, N_MELS] bf16  mel filterbank, split by k parity
    gr:  [128, 2] f32       window ratio hw_hi/hw_lo, n-chunked
    out: [R, N_MELS] f32
    """
    nc = bacc.Bacc(None, target_bir_lowering=False)
    xt_d = nc.declare_dram_parameter("xt", [NQ, 128, R], f16, isOutput=False)
    ce_d = nc.declare_dram_parameter("ce", [2, 128, 128], f16, isOutput=False)
    wo_d = nc.declare_dram_parameter("wo", [NQ, 128, 128], f16, isOutput=False)
    fbt_d = nc.declare_dram_parameter("fbt", [2, 128, N_MELS], bf16, isOutput=False)
    g_d = nc.declare_dram_parameter("gr", [128, 2], f32, isOutput=False)
    out_d = nc.declare_dram_parameter("out", [R, N_MELS], f32, isOutput=True)

    with TileContext(nc) as tc:
        with (
            tc.tile_pool(name="consts", bufs=1) as consts,
            tc.tile_pool(name="xt", bufs=2) as xt_pool,
            tc.tile_pool(name="gx", bufs=2) as gx_pool,
            tc.tile_pool(name="u", bufs=2) as u_pool,
            tc.tile_pool(name="mag", bufs=3) as mag_pool,
            tc.tile_pool(name="fin", bufs=2) as fin_pool,
            tc.tile_pool(name="ps_y", bufs=2, space="PSUM") as ps_y_pool,
            tc.tile_pool(name="ps_m", bufs=2, space="PSUM") as ps_m_pool,
        ):
            ce_sb = consts.tile([128, 2, 128], f16)
            nc.sync.dma_start(out=ce_sb, in_=ce_d.rearrange("c p k -> p c k"))
            wo_sb = consts.tile([128, NQ, 128], f16)
            nc.sync.dma_start(out=wo_sb, in_=wo_d.rearrange("c p k -> p c k"))
            fbt_sb = consts.tile([128, 2, N_MELS], bf16)
            nc.sync.dma_start(out=fbt_sb, in_=fbt_d.rearrange("e j m -> j e m"))
            g_sb = consts.tile([128, 2], f32)
            nc.sync.dma_start(out=g_sb, in_=g_d[:, :])

            for r0, RB in MACROS:
                S = RB // 128  # output slots per macro
                xt_sb = xt_pool.tile([128, NQ, RB], f16, name="xt_sb")
                nc.gpsimd.dma_start(
                    out=xt_sb,
                    in_=xt_d[:, :, r0 : r0 + RB].rearrange("c p r -> p c r"),
                )
                # even-side window+fold: u[c] = x[c] + g[c]*x[c+2]
                # (hw_lo is folded into ce; odd side runs unfolded on the PE)
                gx_sb = gx_pool.tile([128, 2, RB], f16, name="gx_sb")
                u_sb = u_pool.tile([128, 2, RB], f16, name="u_sb")
                for c in range(2):
                    nc.vector.tensor_scalar(
                        gx_sb[:, c], xt_sb[:, c + 2], g_sb[:, c : c + 1],
                        None, mybir.AluOpType.mult,
                    )
                    nc.vector.tensor_add(u_sb[:, c], xt_sb[:, c], gx_sb[:, c])

                e_sb = fin_pool.tile([128, S * N_MELS], i32, tag="e_sb", name="e_sb")
                for g in range(RB // GR):
                    r = slice(g * GR, (g + 1) * GR)
                    # DFT: y[k, r] for even/odd k (f32 PSUM accumulate)
                    y_ps = ps_y_pool.tile([128, 2, GR], f32, name="y_ps")
                    for c in range(2):
                        nc.tensor.matmul(
                            y_ps[:, 0, :], ce_sb[:, c, :], u_sb[:, c, r],
                            start=(c == 0), stop=(c == 1),
                        )
                    for c in range(NQ):
                        nc.tensor.matmul(
                            y_ps[:, 1, :], wo_sb[:, c, :], xt_sb[:, c, r],
                            start=(c == 0), stop=(c == NQ - 1),
                        )
                    # mag = y^2 (fused PSUM -> SBUF bf16)
                    mag_sb = mag_pool.tile([128, 2, GR], bf16, name="mag_sb")
                    nc.scalar.activation(
                        mag_sb, y_ps, mybir.ActivationFunctionType.Square
                    )
                    # mel: mels[r, m] += mag[k, r].T @ fbt[k, m]
                    mels_ps = ps_m_pool.tile([128, (GR // 128) * N_MELS], f32,
                                             name="mels_ps")
                    for j in range(GR // 128):
                        jj = slice(j * 128, (j + 1) * 128)
                        for e in range(2):
                            nc.tensor.matmul(
                                mels_ps[:, j * N_MELS : (j + 1) * N_MELS],
                                mag_sb[:, e, jj], fbt_sb[:, e, :],
                                start=(e == 0), stop=(e == 1),
                            )
                    # exponent bits out of PSUM
                    nc.vector.tensor_scalar(
                        e_sb[:, g * 4 * N_MELS : (g + 1) * 4 * N_MELS],
                        mels_ps.bitcast(i32),
                        23,
                        None,
                        mybir.AluOpType.logical_shift_right,
                    )
                # finalize: floor(log2(m)) = max(bits >> 23, 75) - 127
                ef_sb = fin_pool.tile([128, S * N_MELS], f32, tag="ef_sb", name="ef_sb")
                nc.vector.tensor_copy(ef_sb, e_sb)
                o_sb = fin_pool.tile([128, S * N_MELS], f32, tag="o_sb", name="o_sb")
                nc.vector.tensor_scalar(
                    o_sb, ef_sb, 75.0, 127.0,
                    mybir.AluOpType.max, mybir.AluOpType.subtract,
                )
                # store: rows r0 + p*S + s are partition-contiguous in DRAM
                nc.sync.dma_start(
                    out=out_d[r0 : r0 + RB, :].rearrange("(p j) m -> p (j m)", j=S),
                    in_=o_sb,
                )
    nc.compile()
    return nc


def _prep_weights(filter_banks, hw):
    fb = np.asarray(filter_banks, dtype=np.float32)
    n_mels, n_bins = fb.shape  # (20, 257)
    assert n_mels == N_MELS and n_bins == FRAME // 2 + 1
    assert np.all(fb[:, 0] == 0.0), "parity-fold kernel needs an unused DC bin"

    k_even = np.arange(2, 257, 2)  # 128 bins: 2..256
    k_odd = np.arange(1, 256, 2)  # 128 bins: 1..255
    n256 = np.arange(256, dtype=np.float64)
    n512 = np.arange(512, dtype=np.float64)
    hw64 = np.asarray(hw, dtype=np.float64)
    ce = (hw64[:256, None]
          * np.cos(2.0 * np.pi * np.outer(n256, k_even) / FRAME) / FRAME)
    wo = (hw64[:, None]
          * np.cos(2.0 * np.pi * np.outer(n512, k_odd) / FRAME) / FRAME)
    ce = ce.reshape(2, 128, 128).astype(np.float16)
    wo = wo.reshape(NQ, 128, 128).astype(np.float16)

    fbt = np.empty((2, 128, N_MELS), dtype=ml_dtypes.bfloat16)
    fbt[0] = fb[:, k_even].T
    fbt[1] = fb[:, k_odd].T

    g = (hw64[256:] / hw64[:256]).astype(np.float32)  # [256]
    gr = np.ascontiguousarray(g.reshape(2, 128).T)  # [128, 2]
    return ce, wo, fbt, gr


def _prep_inputs(x):
    """Shard, permute, transpose, cast: per core xt[c, p, r] with the macro-
    local row order r = 128*s + p_out chosen so stores are contiguous."""
    x16 = x.reshape(N_CORES, R, FRAME).astype(np.float16)
    parts = []
    for r0, RB in MACROS:
        S = RB // 128
        blk = x16[:, r0 : r0 + RB, :].reshape(N_CORES, 128, S, FRAME)
        # [core, p, s, n] -> [core, n, s, p] -> [core, NQ, 128, S*128]
        t = blk.transpose(0, 3, 2, 1).reshape(N_CORES, NQ, 128, RB)
        parts.append(t)
    xt = np.concatenate(parts, axis=3)  # [core, NQ, 128, R]
    return np.ascontiguousarray(xt)


_CACHE = {}


def _get_graph():
    if "nc" not in _CACHE:
        _CACHE["nc"] = build_graph()
    return _CACHE["nc"]


def kernel(inputs, filter_banks, hw, _trace=False):
    x = np.ascontiguousarray(np.asarray(inputs, dtype=np.float32))
    assert x.shape == (B, T, FRAME), x.shape
    ce, wo, fbt, gr = _prep_weights(filter_banks, hw)
    xt = _prep_inputs(x)

    nc = _get_graph()
    in_maps = [
        {"xt": xt[i], "ce": ce, "wo": wo, "fbt": fbt, "gr": gr}
        for i in range(N_CORES)
    ]
    res = run_bass_kernel_spmd(
        nc, in_maps, core_ids=list(range(N_CORES)), trace=_trace
    )
    out = np.stack([res.results[i]["out"] for i in range(N_CORES)], axis=0)
    out = out.reshape(B, T, N_MELS, 1).astype(np.float32)
    if _trace:
        kernel._last_result = res
    return out


# revision 18
# speedup vs baseline: 1.3880x; 1.1371x over previous
"""Trainium2 Bass kernel for AudioPreprocessingLayer.

Computes: floor(log2(mel_fb @ (rfft(x*hamming, norm=forward).real ** 2)))
for x of shape (4096, 32, 512), sharded batch-wise across 8 NeuronCores.

Key ideas:
  - rfft(.).real is a matmul with the cosine matrix C[n,k] = cos(2*pi*k*n/512)/512.
  - Parity fold: C[n+256, k] = (-1)^k C[n, k], so the even-k bins need only
    ue[n] = hw[n]x[n] + hw[n+256]x[n+256] and the odd-k bins only
    uo[n] = hw[n]x[n] - hw[n+256]x[n+256] — a 256-long contraction instead
    of 512: the DFT matmul work halves.
  - Window-in-weights: ue = hw_lo * (x_lo + g*x_hi) with g = hw_hi/hw_lo,
    and the outer hw_lo folds into the cosine weights. So the whole
    window+fold is 4 scalar_tensor_tensor ops per macro-block
    (out = (x_hi * ±g) + x_lo, g a per-partition scalar).
  - The host hands the kernel x already TRANSPOSED to [n, r] layout (a pure
    permutation, done during sharding), so no on-chip transpose is needed:
    the DMA-loaded tiles feed the DFT matmul directly with n on partitions.
  - The row order within each DMA macro-block is permuted host-side so the
    OUTPUT rows land partition-contiguous (big store descriptors).
  - fp16 end-to-end for x/u and the windowed cosine weights (better
    precision than a bf16 pipeline and full PE speed); mag/filterbank in
    bf16 (fp16 would flush y^2 subnormals); PSUM accumulation in f32.
  - floor(log2(m)) for positive fp32 m is exactly
    max(bitcast_int32(m) >> 23, 75) - 127   (the max() also maps the
    mels==0 -> eps=2^-52 case to -52 exactly).
"""

import os
import sys

for _p in ("/opt/trn_rl_repo",):
    if _p not in sys.path and os.path.isdir(_p):
        sys.path.append(_p)

import numpy as np
import ml_dtypes

import concourse.bass as bass
from concourse import bacc, mybir
from concourse.tile import TileContext
from concourse.bass_utils import run_bass_kernel_spmd

N_CORES = 8
B, T, FRAME = 4096, 32, 512
R = (B // N_CORES) * T  # 16384 rows of length 512 per core
N_MELS = 20
NQ = FRAME // 128  # 4 n-chunks of the transposed input
GR = 512  # rows per compute group (one PSUM bank of f32)

# DMA macro-blocks (rows): first ones small so the pipeline fills quickly.
MACROS = [(0, 256), (256, 768), (1024, 1024)] + [
    (2048 + 2048 * i, 2048) for i in range(7)
]
assert sum(rb for _, rb in MACROS) == R

f32 = mybir.dt.float32
f16 = mybir.dt.float16
bf16 = mybir.dt.bfloat16
i32 = mybir.dt.int32


def build_graph():
    """SPMD Bass graph for one core's shard.

    xt:  [NQ, 128, R] f16   transposed rows: xt[c, p, r] = x[perm(r), 128c+p]
    ce:  [2, 128, 128] f16  diag(hw_lo) @ cos matrix, even k (2,4,...,256)
    wo:  [NQ, 128, 128] f16 full windowed cos matrix, odd k (1,3,...,255)
    fbt: [2, 128, N_MELS] bf16  mel filterbank, split by k parity
    gr:  [128, 2] f32       window ratio hw_hi/hw_lo, n-chunked
    out: [R, N_MELS] f32
    """
    nc = bacc.Bacc(None, target_bir_lowering=False)
    xt_d = nc.declare_dram_parameter("xt", [NQ, 128, R], f16, isOutput=False)
    ce_d = nc.declare_dram_parameter("ce", [2, 128, 128], f16, isOutput=False)
    wo_d = nc.declare_dram_parameter("wo", [NQ, 128, 128], f16, isOutput=False)
    fbt_d = nc.declare_dram_parameter("fbt", [2, 128, N_MELS], bf16, isOutput=False)
    g_d = nc.declare_dram_parameter("gr", [128, 2], f32, isOutput=False)
    out_d = nc.declare_dram_parameter("out", [R, N_MELS], f32, isOutput=True)

    with TileContext(nc) as tc:
        with (
            tc.tile_pool(name="consts", bufs=1) as consts,
            tc.tile_pool(name="xt", bufs=3) as xt_pool,
            tc.tile_pool(name="gx", bufs=3) as gx_pool,
            tc.tile_pool(name="u", bufs=3) as u_pool,
            tc.tile_pool(name="mag", bufs=3) as mag_pool,
            tc.tile_pool(name="fin", bufs=2) as fin_pool,
            tc.tile_pool(name="ps_y", bufs=3, space="PSUM") as ps_y_pool,
            tc.tile_pool(name="ps_m", bufs=2, space="PSUM") as ps_m_pool,
        ):
            # gr first (unblocks the folds), then matmul weights.
            g_sb = consts.tile([128, 2], f32)
            nc.sync.dma_start(out=g_sb, in_=g_d[:, :])
            ce_sb = consts.tile([128, 2, 128], f16)
            nc.sync.dma_start(out=ce_sb, in_=ce_d.rearrange("c p k -> p c k"))
            wo_sb = consts.tile([128, NQ, 128], f16)
            nc.sync.dma_start(out=wo_sb, in_=wo_d.rearrange("c p k -> p c k"))
            fbt_sb = consts.tile([128, 2, N_MELS], bf16)
            nc.sync.dma_start(out=fbt_sb, in_=fbt_d.rearrange("e j m -> j e m"))

            def emit_load(m):
                """DMA + even-side window/fold for macro m."""
                r0, RB = MACROS[m]
                xt_sb = xt_pool.tile([128, NQ, RB], f16, name="xt_sb")
                q = nc.gpsimd if m % 2 == 0 else nc.sync
                q.dma_start(
                    out=xt_sb,
                    in_=xt_d[:, :, r0 : r0 + RB].rearrange("c p r -> p c r"),
                )
                # u[c] = x[c] + g[c]*x[c+2]  (hw_lo is folded into ce;
                # the odd side runs unfolded on the PE straight from xt)
                gx_sb = gx_pool.tile([128, 2, RB], f16, name="gx_sb")
                u_sb = u_pool.tile([128, 2, RB], f16, name="u_sb")
                for c in range(2):
                    nc.vector.tensor_scalar(
                        gx_sb[:, c], xt_sb[:, c + 2], g_sb[:, c : c + 1],
                        None, mybir.AluOpType.mult,
                    )
                    nc.vector.tensor_add(u_sb[:, c], xt_sb[:, c], gx_sb[:, c])
                return xt_sb, u_sb

            def emit_groups(m, xt_sb, u_sb):
                r0, RB = MACROS[m]
                S = RB // 128  # output slots per macro
                e_sb = fin_pool.tile([128, S * N_MELS], i32, tag="e_sb",
                                     name="e_sb")
                for off in range(0, RB, GR):
                    gr_n = min(GR, RB - off)
                    r = slice(off, off + gr_n)
                    # DFT: y[k, r] for even/odd k (f32 PSUM accumulate)
                    y_ps = ps_y_pool.tile([128, 2, gr_n], f32, name="y_ps")
                    for c in range(2):
                        nc.tensor.matmul(
                            y_ps[:, 0, :], ce_sb[:, c, :], u_sb[:, c, r],
                            start=(c == 0), stop=(c == 1),
                        )
                    for c in range(NQ):
                        nc.tensor.matmul(
                            y_ps[:, 1, :], wo_sb[:, c, :], xt_sb[:, c, r],
                            start=(c == 0), stop=(c == NQ - 1),
                        )
                    # mag = y^2 (fused PSUM -> SBUF bf16)
                    mag_sb = mag_pool.tile([128, 2, gr_n], bf16, name="mag_sb")
                    nc.scalar.activation(
                        mag_sb, y_ps, mybir.ActivationFunctionType.Square
                    )
                    # mel: mels[r, m] += mag[k, r].T @ fbt[k, m]
                    nj = gr_n // 128
                    mels_ps = ps_m_pool.tile([128, nj * N_MELS], f32,
                                             name="mels_ps")
                    for j in range(nj):
                        jj = slice(j * 128, (j + 1) * 128)
                        for e in range(2):
                            nc.tensor.matmul(
                                mels_ps[:, j * N_MELS : (j + 1) * N_MELS],
                                mag_sb[:, e, jj], fbt_sb[:, e, :],
                                start=(e == 0), stop=(e == 1),
                            )
                    # exponent bits out of PSUM
                    s0 = off // 128  # first slot of this group
                    nc.vector.tensor_scalar(
                        e_sb[:, s0 * N_MELS : (s0 + nj) * N_MELS],
                        mels_ps.bitcast(i32),
                        23,
                        None,
                        mybir.AluOpType.logical_shift_right,
                    )
                # finalize: floor(log2(m)) = max(bits >> 23, 75) - 127
                ef_sb = fin_pool.tile([128, S * N_MELS], f32, tag="ef_sb",
                                      name="ef_sb")
                nc.vector.tensor_copy(ef_sb, e_sb)
                o_sb = fin_pool.tile([128, S * N_MELS], f32, tag="o_sb",
                                     name="o_sb")
                nc.vector.tensor_scalar(
                    o_sb, ef_sb, 75.0, 127.0,
                    mybir.AluOpType.max, mybir.AluOpType.subtract,
                )
                # store: rows r0 + p*S + s are partition-contiguous in DRAM
                q = nc.sync if m % 2 == 0 else nc.gpsimd
                q.dma_start(
                    out=out_d[r0 : r0 + RB, :].rearrange(
                        "(p j) m -> p (j m)", j=S
                    ),
                    in_=o_sb,
                )

            # software pipeline: load macro m+1 (DMA + DVE folds) before
            # emitting macro m's matmul groups, so the DVE FIFO never parks
            # next macro's folds behind this macro's exponent shifts.
            pending = {0: emit_load(0)}
            for m in range(len(MACROS)):
                if m + 1 < len(MACROS):
                    pending[m + 1] = emit_load(m + 1)
                emit_groups(m, *pending.pop(m))
    nc.compile()
    return nc


def _prep_weights(filter_banks, hw):
    fb = np.asarray(filter_banks, dtype=np.float32)
    n_mels, n_bins = fb.shape  # (20, 257)
    assert n_mels == N_MELS and n_bins == FRAME // 2 + 1
    assert np.all(fb[:, 0] == 0.0), "parity-fold kernel needs an unused DC bin"

    k_even = np.arange(2, 257, 2)  # 128 bins: 2..256
    k_odd = np.arange(1, 256, 2)  # 128 bins: 1..255
    n256 = np.arange(256, dtype=np.float64)
    n512 = np.arange(512, dtype=np.float64)
    hw64 = np.asarray(hw, dtype=np.float64)
    ce = (hw64[:256, None]
          * np.cos(2.0 * np.pi * np.outer(n256, k_even) / FRAME) / FRAME)
    wo = (hw64[:, None]
          * np.cos(2.0 * np.pi * np.outer(n512, k_odd) / FRAME) / FRAME)
    ce = ce.reshape(2, 128, 128).astype(np.float16)
    wo = wo.reshape(NQ, 128, 128).astype(np.float16)

    fbt = np.empty((2, 128, N_MELS), dtype=ml_dtypes.bfloat16)
    fbt[0] = fb[:, k_even].T
    fbt[1] = fb[:, k_odd].T

    g = (hw64[256:] / hw64[:256]).astype(np.float32)  # [256]
    gr = np.ascontiguousarray(g.reshape(2, 128).T)  # [128, 2]
    return ce, wo, fbt, gr


def _prep_inputs(x):
    """Shard, permute, transpose, cast: per core xt[c, p, r] with the macro-
    local row order r = 128*s + p_out chosen so stores are contiguous."""
    x16 = x.reshape(N_CORES, R, FRAME).astype(np.float16)
    parts = []
    for r0, RB in MACROS:
        S = RB // 128
        blk = x16[:, r0 : r0 + RB, :].reshape(N_CORES, 128, S, FRAME)
        # [core, p, s, n] -> [core, n, s, p] -> [core, NQ, 128, S*128]
        t = blk.transpose(0, 3, 2, 1).reshape(N_CORES, NQ, 128, RB)
        parts.append(t)
    xt = np.concatenate(parts, axis=3)  # [core, NQ, 128, R]
    return np.ascontiguousarray(xt)


_CACHE = {}


def _get_graph():
    if "nc" not in _CACHE:
        _CACHE["nc"] = build_graph()
    return _CACHE["nc"]


def kernel(inputs, filter_banks, hw, _trace=False):
    x = np.ascontiguousarray(np.asarray(inputs, dtype=np.float32))
    assert x.shape == (B, T, FRAME), x.shape
    ce, wo, fbt, gr = _prep_weights(filter_banks, hw)
    xt = _prep_inputs(x)

    nc = _get_graph()
    in_maps = [
        {"xt": xt[i], "ce": ce, "wo": wo, "fbt": fbt, "gr": gr}
        for i in range(N_CORES)
    ]
    res = run_bass_kernel_spmd(
        nc, in_maps, core_ids=list(range(N_CORES)), trace=_trace
    )
    out = np.stack([res.results[i]["out"] for i in range(N_CORES)], axis=0)
    out = out.reshape(B, T, N_MELS, 1).astype(np.float32)
    if _trace:
        kernel._last_result = res
    return out


# revision 23
# speedup vs baseline: 1.6349x; 1.1779x over previous
"""Trainium2 Bass kernel for AudioPreprocessingLayer.

Computes: floor(log2(mel_fb @ (rfft(x*hamming, norm=forward).real ** 2)))
for x of shape (4096, 32, 512), sharded batch-wise across 8 NeuronCores.

Key ideas:
  - rfft(.).real is a matmul with the cosine matrix C[n,k] = cos(2*pi*k*n/512)/512.
  - Parity fold: C[n+256, k] = (-1)^k C[n, k], so the even-k bins need only
    ue[n] = hw[n]x[n] + hw[n+256]x[n+256] and the odd-k bins only
    uo[n] = hw[n]x[n] - hw[n+256]x[n+256] — a 256-long contraction instead
    of 512: the DFT matmul work halves.
  - Window-in-weights: ue = hw_lo * (x_lo + g*x_hi) with g = hw_hi/hw_lo,
    and the outer hw_lo folds into the cosine weights. So the whole
    window+fold is 4 scalar_tensor_tensor ops per macro-block
    (out = (x_hi * ±g) + x_lo, g a per-partition scalar).
  - The host hands the kernel x already TRANSPOSED to [n, r] layout (a pure
    permutation, done during sharding), so no on-chip transpose is needed:
    the DMA-loaded tiles feed the DFT matmul directly with n on partitions.
  - The row order within each DMA macro-block is permuted host-side so the
    OUTPUT rows land partition-contiguous (big store descriptors).
  - fp16 end-to-end for x/u and the windowed cosine weights (better
    precision than a bf16 pipeline and full PE speed); mag/filterbank in
    bf16 (fp16 would flush y^2 subnormals); PSUM accumulation in f32.
  - floor(log2(m)) for positive fp32 m is exactly
    max(bitcast_int32(m) >> 23, 75) - 127   (the max() also maps the
    mels==0 -> eps=2^-52 case to -52 exactly).
"""

import os
import sys

for _p in ("/opt/trn_rl_repo",):
    if _p not in sys.path and os.path.isdir(_p):
        sys.path.append(_p)

import numpy as np
import ml_dtypes

import concourse.bass as bass
from concourse import bacc, mybir
from concourse.tile import TileContext
from concourse.bass_utils import run_bass_kernel_spmd

N_CORES = 8
B, T, FRAME = 4096, 32, 512
R = (B // N_CORES) * T  # 16384 rows of length 512 per core
N_MELS = 20
NQ = FRAME // 128  # 4 n-chunks of the transposed input
GR = 512  # rows per compute group (one PSUM bank of f32)

# DMA macro-blocks (rows): first ones small so the pipeline fills quickly.
MACROS = [(0, 256), (256, 768), (1024, 1024)] + [
    (2048 + 2048 * i, 2048) for i in range(7)
]
assert sum(rb for _, rb in MACROS) == R

f32 = mybir.dt.float32
f16 = mybir.dt.float16
bf16 = mybir.dt.bfloat16
i32 = mybir.dt.int32


def build_graph():
    """SPMD Bass graph for one core's shard.

    xt:  [NQ, 128, R] f16   transposed rows: xt[c, p, r] = x[perm(r), 128c+p]
    ce:  [2, 128, 128] f16  diag(hw_lo) @ cos matrix, even k (2,4,...,256)
    cef: [NQ, 128, 128] f16 full windowed cos matrix, even k (macro-0 path)
    wo:  [NQ, 128, 128] f16 full windowed cos matrix, odd k (1,3,...,255)
    fbt: [2, 128, N_MELS] bf16  mel filterbank * 2^-75, split by k parity
         (the 2^-75 bias makes f32 subnormal flush implement the eps clamp:
          floor(log2(mels)) = (expbits(mels * 2^-75) >> 23) - 52, exact for
          mels > 2^-51, and the mels==0 -> eps path lands on -52 via the
          zero/subnormal exponent field)
    gr:  [128, 2] f32       window ratio hw_hi/hw_lo, n-chunked
    out: [R, N_MELS] f32
    """
    nc = bacc.Bacc(None, target_bir_lowering=False)
    xt_d = nc.declare_dram_parameter("xt", [NQ, 128, R], f16, isOutput=False)
    ce_d = nc.declare_dram_parameter("ce", [2, 128, 128], f16, isOutput=False)
    cef_d = nc.declare_dram_parameter("cef", [NQ, 128, 128], f16, isOutput=False)
    wo_d = nc.declare_dram_parameter("wo", [NQ, 128, 128], f16, isOutput=False)
    fbt_d = nc.declare_dram_parameter("fbt", [2, 128, N_MELS], bf16, isOutput=False)
    g_d = nc.declare_dram_parameter("gr", [128, 2], f32, isOutput=False)
    out_d = nc.declare_dram_parameter("out", [R, N_MELS], f32, isOutput=True)

    with TileContext(nc) as tc:
        with (
            tc.tile_pool(name="consts", bufs=1) as consts,
            tc.tile_pool(name="xt", bufs=3) as xt_pool,
            tc.tile_pool(name="gx", bufs=3) as gx_pool,
            tc.tile_pool(name="u", bufs=3) as u_pool,
            tc.tile_pool(name="mag", bufs=3) as mag_pool,
            tc.tile_pool(name="fin", bufs=2) as fin_pool,
            tc.tile_pool(name="ps_y", bufs=3, space="PSUM") as ps_y_pool,
            tc.tile_pool(name="ps_m", bufs=2, space="PSUM") as ps_m_pool,
        ):
            # gr first (unblocks the folds), then matmul weights.
            g_sb = consts.tile([128, 2], f32)
            nc.sync.dma_start(out=g_sb, in_=g_d[:, :])
            ce_sb = consts.tile([128, 2, 128], f16)
            nc.sync.dma_start(out=ce_sb, in_=ce_d.rearrange("c p k -> p c k"))
            wo_sb = consts.tile([128, NQ, 128], f16)
            nc.sync.dma_start(out=wo_sb, in_=wo_d.rearrange("c p k -> p c k"))
            cef_sb = consts.tile([128, NQ, 128], f16)
            nc.sync.dma_start(out=cef_sb, in_=cef_d.rearrange("c p k -> p c k"))
            fbt_sb = consts.tile([128, 2, N_MELS], bf16)
            nc.sync.dma_start(out=fbt_sb, in_=fbt_d.rearrange("e j m -> j e m"))

            def emit_load(m):
                """DMA + even-side window/fold for macro m."""
                r0, RB = MACROS[m]
                xt_sb = xt_pool.tile([128, NQ, RB], f16, name="xt_sb")
                q = nc.gpsimd if m % 2 == 0 else nc.sync
                q.dma_start(
                    out=xt_sb,
                    in_=xt_d[:, :, r0 : r0 + RB].rearrange("c p r -> p c r"),
                )
                if m == 0:
                    # macro 0 computes its even side unfolded on the PE (via
                    # cef), so nothing blocks on the DVE during pipe fill.
                    return xt_sb, None
                # u[c] = x[c] + g[c]*x[c+2]  (hw_lo is folded into ce;
                # the odd side runs unfolded on the PE straight from xt)
                gx_sb = gx_pool.tile([128, 2, RB], f16, name="gx_sb")
                u_sb = u_pool.tile([128, 2, RB], f16, name="u_sb")
                for c in range(2):
                    nc.vector.tensor_scalar(
                        gx_sb[:, c], xt_sb[:, c + 2], g_sb[:, c : c + 1],
                        None, mybir.AluOpType.mult,
                    )
                    nc.vector.tensor_add(u_sb[:, c], xt_sb[:, c], gx_sb[:, c])
                return xt_sb, u_sb

            def emit_groups(m, xt_sb, u_sb):
                r0, RB = MACROS[m]
                S = RB // 128  # output slots per macro
                mels_ps = ps_m_pool.tile([128, S * N_MELS], f32, name="mels_ps")
                for off in range(0, RB, GR):
                    gr_n = min(GR, RB - off)
                    r = slice(off, off + gr_n)
                    # DFT: y[k, r] for even/odd k (f32 PSUM accumulate);
                    # odd first — it reads xt directly, no DVE dependency.
                    y_ps = ps_y_pool.tile([128, 2, gr_n], f32, name="y_ps")
                    for c in range(NQ):
                        nc.tensor.matmul(
                            y_ps[:, 1, :], wo_sb[:, c, :], xt_sb[:, c, r],
                            start=(c == 0), stop=(c == NQ - 1),
                        )
                    if u_sb is None:
                        for c in range(NQ):
                            nc.tensor.matmul(
                                y_ps[:, 0, :], cef_sb[:, c, :], xt_sb[:, c, r],
                                start=(c == 0), stop=(c == NQ - 1),
                            )
                    else:
                        for c in range(2):
                            nc.tensor.matmul(
                                y_ps[:, 0, :], ce_sb[:, c, :], u_sb[:, c, r],
                                start=(c == 0), stop=(c == 1),
                            )
                    # mag = y^2 (fused PSUM -> SBUF bf16)
                    mag_sb = mag_pool.tile([128, 2, gr_n], bf16, name="mag_sb")
                    nc.scalar.activation(
                        mag_sb, y_ps, mybir.ActivationFunctionType.Square
                    )
                    # mel: mels[r, m] += mag[k, r].T @ fbt[k, m]
                    # (a whole macro's mels fit one PSUM bank)
                    for j in range(gr_n // 128):
                        jj = slice(j * 128, (j + 1) * 128)
                        s = off // 128 + j
                        for e in range(2):
                            nc.tensor.matmul(
                                mels_ps[:, s * N_MELS : (s + 1) * N_MELS],
                                mag_sb[:, e, jj], fbt_sb[:, e, :],
                                start=(e == 0), stop=(e == 1),
                            )
                # finalize: floor(log2(mels)) = expbits(mels * 2^-75) - 52
                e_sb = fin_pool.tile([128, S * N_MELS], i32, tag="e_sb",
                                     name="e_sb")
                nc.vector.tensor_scalar(
                    e_sb,
                    mels_ps.bitcast(i32),
                    23,
                    None,
                    mybir.AluOpType.logical_shift_right,
                )
                o_sb = fin_pool.tile([128, S * N_MELS], f32, tag="o_sb",
                                     name="o_sb")
                nc.vector.tensor_scalar_sub(o_sb, e_sb, 52.0)
                # store: rows r0 + p*S + s are partition-contiguous in DRAM
                q = nc.sync if m % 2 == 0 else nc.gpsimd
                q.dma_start(
                    out=out_d[r0 : r0 + RB, :].rearrange(
                        "(p j) m -> p (j m)", j=S
                    ),
                    in_=o_sb,
                )

            # software pipeline: load macro m+1 (DMA + DVE folds) before
            # emitting macro m's matmul groups, so the DVE FIFO never parks
            # next macro's folds behind this macro's exponent shifts.
            pending = {0: emit_load(0)}
            for m in range(len(MACROS)):
                if m + 1 < len(MACROS):
                    pending[m + 1] = emit_load(m + 1)
                emit_groups(m, *pending.pop(m))
    nc.compile()
    return nc


def _prep_weights(filter_banks, hw):
    fb = np.asarray(filter_banks, dtype=np.float32)
    n_mels, n_bins = fb.shape  # (20, 257)
    assert n_mels == N_MELS and n_bins == FRAME // 2 + 1
    assert np.all(fb[:, 0] == 0.0), "parity-fold kernel needs an unused DC bin"

    k_even = np.arange(2, 257, 2)  # 128 bins: 2..256
    k_odd = np.arange(1, 256, 2)  # 128 bins: 1..255
    n256 = np.arange(256, dtype=np.float64)
    n512 = np.arange(512, dtype=np.float64)
    hw64 = np.asarray(hw, dtype=np.float64)
    ce = (hw64[:256, None]
          * np.cos(2.0 * np.pi * np.outer(n256, k_even) / FRAME) / FRAME)
    cef = (hw64[:, None]
           * np.cos(2.0 * np.pi * np.outer(n512, k_even) / FRAME) / FRAME)
    wo = (hw64[:, None]
          * np.cos(2.0 * np.pi * np.outer(n512, k_odd) / FRAME) / FRAME)
    ce = ce.reshape(2, 128, 128).astype(np.float16)
    cef = cef.reshape(NQ, 128, 128).astype(np.float16)
    wo = wo.reshape(NQ, 128, 128).astype(np.float16)

    # 2^-75 bias: the on-device eps clamp comes from subnormal flush of
    # mels * 2^-75 (see build_graph docstring). Exact power-of-2 scale.
    fbt = np.empty((2, 128, N_MELS), dtype=ml_dtypes.bfloat16)
    fbt[0] = (fb[:, k_even] * np.float32(2.0**-75)).T
    fbt[1] = (fb[:, k_odd] * np.float32(2.0**-75)).T

    g = (hw64[256:] / hw64[:256]).astype(np.float32)  # [256]
    gr = np.ascontiguousarray(g.reshape(2, 128).T)  # [128, 2]
    return ce, cef, wo, fbt, gr


def _prep_inputs(x):
    """Shard, permute, transpose, cast: per core xt[c, p, r] with the macro-
    local row order r = 128*s + p_out chosen so stores are contiguous."""
    x16 = x.reshape(N_CORES, R, FRAME).astype(np.float16)
    parts = []
    for r0, RB in MACROS:
        S = RB // 128
        blk = x16[:, r0 : r0 + RB, :].reshape(N_CORES, 128, S, FRAME)
        # [core, p, s, n] -> [core, n, s, p] -> [core, NQ, 128, S*128]
        t = blk.transpose(0, 3, 2, 1).reshape(N_CORES, NQ, 128, RB)
        parts.append(t)
    xt = np.concatenate(parts, axis=3)  # [core, NQ, 128, R]
    return np.ascontiguousarray(xt)


_CACHE = {}


def _get_graph():
    if "nc" not in _CACHE:
        _CACHE["nc"] = build_graph()
    return _CACHE["nc"]


def kernel(inputs, filter_banks, hw, _trace=False):
    x = np.ascontiguousarray(np.asarray(inputs, dtype=np.float32))
    assert x.shape == (B, T, FRAME), x.shape
    ce, cef, wo, fbt, gr = _prep_weights(filter_banks, hw)
    xt = _prep_inputs(x)

    nc = _get_graph()
    in_maps = [
        {"xt": xt[i], "ce": ce, "cef": cef, "wo": wo, "fbt": fbt, "gr": gr}
        for i in range(N_CORES)
    ]
    res = run_bass_kernel_spmd(
        nc, in_maps, core_ids=list(range(N_CORES)), trace=_trace
    )
    out = np.stack([res.results[i]["out"] for i in range(N_CORES)], axis=0)
    out = out.reshape(B, T, N_MELS, 1).astype(np.float32)
    if _trace:
        kernel._last_result = res
    return out


# revision 28
# speedup vs baseline: 1.6881x; 1.0325x over previous
"""Trainium2 Bass kernel for AudioPreprocessingLayer.

Computes: floor(log2(mel_fb @ (rfft(x*hamming, norm=forward).real ** 2)))
for x of shape (4096, 32, 512), sharded batch-wise across 8 NeuronCores.

Key ideas:
  - rfft(.).real is a matmul with the cosine matrix C[n,k] = cos(2*pi*k*n/512)/512.
  - Parity fold: C[n+256, k] = (-1)^k C[n, k], so the even-k bins need only
    ue[n] = hw[n]x[n] + hw[n+256]x[n+256] and the odd-k bins only
    uo[n] = hw[n]x[n] - hw[n+256]x[n+256] — a 256-long contraction instead
    of 512: the DFT matmul work halves.
  - Window-in-weights: ue = hw_lo * (x_lo + g*x_hi) with g = hw_hi/hw_lo,
    and the outer hw_lo folds into the cosine weights. So the whole
    window+fold is 4 scalar_tensor_tensor ops per macro-block
    (out = (x_hi * ±g) + x_lo, g a per-partition scalar).
  - The host hands the kernel x already TRANSPOSED to [n, r] layout (a pure
    permutation, done during sharding), so no on-chip transpose is needed:
    the DMA-loaded tiles feed the DFT matmul directly with n on partitions.
  - The row order within each DMA macro-block is permuted host-side so the
    OUTPUT rows land partition-contiguous (big store descriptors).
  - fp16 end-to-end for x/u and the windowed cosine weights (better
    precision than a bf16 pipeline and full PE speed); mag/filterbank in
    bf16 (fp16 would flush y^2 subnormals); PSUM accumulation in f32.
  - floor(log2(m)) for positive fp32 m is exactly
    max(bitcast_int32(m) >> 23, 75) - 127   (the max() also maps the
    mels==0 -> eps=2^-52 case to -52 exactly).
"""

import os
import sys

for _p in ("/opt/trn_rl_repo",):
    if _p not in sys.path and os.path.isdir(_p):
        sys.path.append(_p)

import numpy as np
import ml_dtypes

import concourse.bass as bass
from concourse import bacc, mybir
from concourse.tile import TileContext
from concourse.bass_utils import run_bass_kernel_spmd

N_CORES = 8
B, T, FRAME = 4096, 32, 512
R = (B // N_CORES) * T  # 16384 rows of length 512 per core
N_MELS = 20
NQ = FRAME // 128  # 4 n-chunks of the transposed input
GR = 512  # rows per compute group (one PSUM bank of f32)

# DMA macro-blocks (rows): small first blocks so the pipeline fills quickly,
# and a smaller last block so the drain tail is short.
MACROS = [(0, 256), (256, 768), (1024, 1024)] + [
    (2048 + 2048 * i, 2048) for i in range(6)
] + [(14336, 1024), (15360, 1024)]
assert sum(rb for _, rb in MACROS) == R

f32 = mybir.dt.float32
f16 = mybir.dt.float16
bf16 = mybir.dt.bfloat16
i32 = mybir.dt.int32


def build_graph():
    """SPMD Bass graph for one core's shard.

    xt:  [NQ, 128, R] f16   transposed rows: xt[c, p, r] = x[perm(r), 128c+p]
    ce:  [2, 128, 128] f16  diag(hw_lo) @ cos matrix, even k (2,4,...,256)
    cef: [NQ, 128, 128] f16 full windowed cos matrix, even k (macro-0 path)
    wo:  [NQ, 128, 128] f16 full windowed cos matrix, odd k (1,3,...,255)
    fbt: [2, 128, N_MELS] bf16  mel filterbank * 2^-75, split by k parity
         (the 2^-75 bias makes f32 subnormal flush implement the eps clamp:
          floor(log2(mels)) = (expbits(mels * 2^-75) >> 23) - 52, exact for
          mels > 2^-51, and the mels==0 -> eps path lands on -52 via the
          zero/subnormal exponent field)
    gr:  [128, 2] f32       window ratio hw_hi/hw_lo, n-chunked
    out: [R, N_MELS] f32
    """
    nc = bacc.Bacc(None, target_bir_lowering=False)
    xt_d = nc.declare_dram_parameter("xt", [NQ, 128, R], f16, isOutput=False)
    ce_d = nc.declare_dram_parameter("ce", [2, 128, 128], f16, isOutput=False)
    cef_d = nc.declare_dram_parameter("cef", [NQ, 128, 128], f16, isOutput=False)
    wo_d = nc.declare_dram_parameter("wo", [NQ, 128, 128], f16, isOutput=False)
    fbt_d = nc.declare_dram_parameter("fbt", [2, 128, N_MELS], bf16, isOutput=False)
    g_d = nc.declare_dram_parameter("gr", [128, 2], f32, isOutput=False)
    out_d = nc.declare_dram_parameter("out", [R, N_MELS], bf16, isOutput=True)

    with TileContext(nc) as tc:
        with (
            tc.tile_pool(name="consts", bufs=1) as consts,
            tc.tile_pool(name="xt", bufs=3) as xt_pool,
            tc.tile_pool(name="gx", bufs=3) as gx_pool,
            tc.tile_pool(name="u", bufs=3) as u_pool,
            tc.tile_pool(name="mag", bufs=3) as mag_pool,
            tc.tile_pool(name="fin", bufs=2) as fin_pool,
            tc.tile_pool(name="ps_y", bufs=3, space="PSUM") as ps_y_pool,
            tc.tile_pool(name="ps_m", bufs=2, space="PSUM") as ps_m_pool,
        ):
            # weights go on the (otherwise idle) Act queue so their descriptor
            # generation never delays the macro input DMAs on sync/gpsimd.
            g_sb = consts.tile([128, 2], f32)
            nc.scalar.dma_start(out=g_sb, in_=g_d[:, :])
            wo_sb = consts.tile([128, NQ, 128], f16)
            nc.scalar.dma_start(out=wo_sb, in_=wo_d.rearrange("c p k -> p c k"))
            cef_sb = consts.tile([128, NQ, 128], f16)
            nc.scalar.dma_start(out=cef_sb, in_=cef_d.rearrange("c p k -> p c k"))
            ce_sb = consts.tile([128, 2, 128], f16)
            nc.scalar.dma_start(out=ce_sb, in_=ce_d.rearrange("c p k -> p c k"))
            fbt_sb = consts.tile([128, 2, N_MELS], bf16)
            nc.scalar.dma_start(out=fbt_sb, in_=fbt_d.rearrange("e j m -> j e m"))

            def emit_load(m):
                """DMA + even-side window/fold for macro m."""
                r0, RB = MACROS[m]
                xt_sb = xt_pool.tile([128, NQ, RB], f16, name="xt_sb")
                q = nc.gpsimd if m % 2 == 0 else nc.sync
                q.dma_start(
                    out=xt_sb,
                    in_=xt_d[:, :, r0 : r0 + RB].rearrange("c p r -> p c r"),
                )
                if m == 0:
                    # macro 0 computes its even side unfolded on the PE (via
                    # cef), so nothing blocks on the DVE during pipe fill.
                    return xt_sb, None
                # u[c] = x[c] + g[c]*x[c+2]  (hw_lo is folded into ce;
                # the odd side runs unfolded on the PE straight from xt)
                gx_sb = gx_pool.tile([128, 2, RB], f16, name="gx_sb")
                u_sb = u_pool.tile([128, 2, RB], f16, name="u_sb")
                for c in range(2):
                    nc.vector.tensor_scalar(
                        gx_sb[:, c], xt_sb[:, c + 2], g_sb[:, c : c + 1],
                        None, mybir.AluOpType.mult,
                    )
                    nc.vector.tensor_add(u_sb[:, c], xt_sb[:, c], gx_sb[:, c])
                return xt_sb, u_sb

            def emit_groups(m, xt_sb, u_sb):
                r0, RB = MACROS[m]
                S = RB // 128  # output slots per macro
                mels_ps = ps_m_pool.tile([128, S * N_MELS], f32, name="mels_ps")
                for off in range(0, RB, GR):
                    gr_n = min(GR, RB - off)
                    r = slice(off, off + gr_n)
                    # DFT: y[k, r] for even/odd k (f32 PSUM accumulate);
                    # odd first — it reads xt directly, no DVE dependency.
                    y_ps = ps_y_pool.tile([128, 2, gr_n], f32, name="y_ps")
                    for c in range(NQ):
                        nc.tensor.matmul(
                            y_ps[:, 1, :], wo_sb[:, c, :], xt_sb[:, c, r],
                            start=(c == 0), stop=(c == NQ - 1),
                        )
                    if u_sb is None:
                        for c in range(NQ):
                            nc.tensor.matmul(
                                y_ps[:, 0, :], cef_sb[:, c, :], xt_sb[:, c, r],
                                start=(c == 0), stop=(c == NQ - 1),
                            )
                    else:
                        for c in range(2):
                            nc.tensor.matmul(
                                y_ps[:, 0, :], ce_sb[:, c, :], u_sb[:, c, r],
                                start=(c == 0), stop=(c == 1),
                            )
                    # mag = y^2 (fused PSUM -> SBUF bf16)
                    mag_sb = mag_pool.tile([128, 2, gr_n], bf16, name="mag_sb")
                    nc.scalar.activation(
                        mag_sb, y_ps, mybir.ActivationFunctionType.Square
                    )
                    # mel: mels[r, m] += mag[k, r].T @ fbt[k, m]
                    # (a whole macro's mels fit one PSUM bank)
                    for j in range(gr_n // 128):
                        jj = slice(j * 128, (j + 1) * 128)
                        s = off // 128 + j
                        for e in range(2):
                            nc.tensor.matmul(
                                mels_ps[:, s * N_MELS : (s + 1) * N_MELS],
                                mag_sb[:, e, jj], fbt_sb[:, e, :],
                                start=(e == 0), stop=(e == 1),
                            )
                # finalize: floor(log2(mels)) = expbits(mels * 2^-75) - 52
                e_sb = fin_pool.tile([128, S * N_MELS], i32, tag="e_sb",
                                     name="e_sb")
                nc.vector.tensor_scalar(
                    e_sb,
                    mels_ps.bitcast(i32),
                    23,
                    None,
                    mybir.AluOpType.logical_shift_right,
                )
                o_sb = fin_pool.tile([128, S * N_MELS], bf16, tag="o_sb",
                                     name="o_sb")
                nc.vector.tensor_scalar_sub(o_sb, e_sb, 52.0)
                # store: rows r0 + p*S + s are partition-contiguous in DRAM
                q = nc.sync if m % 2 == 0 else nc.gpsimd
                q.dma_start(
                    out=out_d[r0 : r0 + RB, :].rearrange(
                        "(p j) m -> p (j m)", j=S
                    ),
                    in_=o_sb,
                )

            # software pipeline: load macro m+1 (DMA + DVE folds) before
            # emitting macro m's matmul groups, so the DVE FIFO never parks
            # next macro's folds behind this macro's exponent shifts.
            pending = {0: emit_load(0)}
            for m in range(len(MACROS)):
                if m + 1 < len(MACROS):
                    pending[m + 1] = emit_load(m + 1)
                emit_groups(m, *pending.pop(m))
    nc.compile()
    return nc


def _prep_weights(filter_banks, hw):
    fb = np.asarray(filter_banks, dtype=np.float32)
    n_mels, n_bins = fb.shape  # (20, 257)
    assert n_mels == N_MELS and n_bins == FRAME // 2 + 1
    assert np.all(fb[:, 0] == 0.0), "parity-fold kernel needs an unused DC bin"

    k_even = np.arange(2, 257, 2)  # 128 bins: 2..256
    k_odd = np.arange(1, 256, 2)  # 128 bins: 1..255
    n256 = np.arange(256, dtype=np.float64)
    n512 = np.arange(512, dtype=np.float64)
    hw64 = np.asarray(hw, dtype=np.float64)
    ce = (hw64[:256, None]
          * np.cos(2.0 * np.pi * np.outer(n256, k_even) / FRAME) / FRAME)
    cef = (hw64[:, None]
           * np.cos(2.0 * np.pi * np.outer(n512, k_even) / FRAME) / FRAME)
    wo = (hw64[:, None]
          * np.cos(2.0 * np.pi * np.outer(n512, k_odd) / FRAME) / FRAME)
    ce = ce.reshape(2, 128, 128).astype(np.float16)
    cef = cef.reshape(NQ, 128, 128).astype(np.float16)
    wo = wo.reshape(NQ, 128, 128).astype(np.float16)

    # 2^-75 bias: the on-device eps clamp comes from subnormal flush of
    # mels * 2^-75 (see build_graph docstring). Exact power-of-2 scale.
    fbt = np.empty((2, 128, N_MELS), dtype=ml_dtypes.bfloat16)
    fbt[0] = (fb[:, k_even] * np.float32(2.0**-75)).T
    fbt[1] = (fb[:, k_odd] * np.float32(2.0**-75)).T

    g = (hw64[256:] / hw64[:256]).astype(np.float32)  # [256]
    gr = np.ascontiguousarray(g.reshape(2, 128).T)  # [128, 2]
    return ce, cef, wo, fbt, gr


def _prep_inputs(x):
    """Shard, permute, transpose, cast: per core xt[c, p, r] with the macro-
    local row order r = 128*s + p_out chosen so stores are contiguous."""
    x16 = x.reshape(N_CORES, R, FRAME).astype(np.float16)
    parts = []
    for r0, RB in MACROS:
        S = RB // 128
        blk = x16[:, r0 : r0 + RB, :].reshape(N_CORES, 128, S, FRAME)
        # [core, p, s, n] -> [core, n, s, p] -> [core, NQ, 128, S*128]
        t = blk.transpose(0, 3, 2, 1).reshape(N_CORES, NQ, 128, RB)
        parts.append(t)
    xt = np.concatenate(parts, axis=3)  # [core, NQ, 128, R]
    return np.ascontiguousarray(xt)


_CACHE = {}


def _get_graph():
    if "nc" not in _CACHE:
        _CACHE["nc"] = build_graph()
    return _CACHE["nc"]


def kernel(inputs, filter_banks, hw, _trace=False):
    x = np.ascontiguousarray(np.asarray(inputs, dtype=np.float32))
    assert x.shape == (B, T, FRAME), x.shape
    ce, cef, wo, fbt, gr = _prep_weights(filter_banks, hw)
    xt = _prep_inputs(x)

    nc = _get_graph()
    in_maps = [
        {"xt": xt[i], "ce": ce, "cef": cef, "wo": wo, "fbt": fbt, "gr": gr}
        for i in range(N_CORES)
    ]
    res = run_bass_kernel_spmd(
        nc, in_maps, core_ids=list(range(N_CORES)), trace=_trace
    )
    out = np.stack(
        [np.asarray(res.results[i]["out"]) for i in range(N_CORES)], axis=0
    )
    # bf16 -> f32 is exact for these small-integer outputs
    out = out.astype(np.float32).reshape(B, T, N_MELS, 1)
    if _trace:
        kernel._last_result = res
    return out


# revision 29
# speedup vs baseline: 1.7220x; 1.0201x over previous
"""Trainium2 Bass kernel for AudioPreprocessingLayer.

Computes: floor(log2(mel_fb @ (rfft(x*hamming, norm=forward).real ** 2)))
for x of shape (4096, 32, 512), sharded batch-wise across 8 NeuronCores.

Key ideas:
  - rfft(.).real is a matmul with the cosine matrix C[n,k] = cos(2*pi*k*n/512)/512.
  - Parity fold: C[n+256, k] = (-1)^k C[n, k], so the even-k bins need only
    ue[n] = hw[n]x[n] + hw[n+256]x[n+256] and the odd-k bins only
    uo[n] = hw[n]x[n] - hw[n+256]x[n+256] — a 256-long contraction instead
    of 512: the DFT matmul work halves.
  - Window-in-weights: ue = hw_lo * (x_lo + g*x_hi) with g = hw_hi/hw_lo,
    and the outer hw_lo folds into the cosine weights. So the whole
    window+fold is 4 scalar_tensor_tensor ops per macro-block
    (out = (x_hi * ±g) + x_lo, g a per-partition scalar).
  - The host hands the kernel x already TRANSPOSED to [n, r] layout (a pure
    permutation, done during sharding), so no on-chip transpose is needed:
    the DMA-loaded tiles feed the DFT matmul directly with n on partitions.
  - The row order within each DMA macro-block is permuted host-side so the
    OUTPUT rows land partition-contiguous (big store descriptors).
  - fp16 end-to-end for x/u and the windowed cosine weights (better
    precision than a bf16 pipeline and full PE speed); mag/filterbank in
    bf16 (fp16 would flush y^2 subnormals); PSUM accumulation in f32.
  - floor(log2(m)) for positive fp32 m is exactly
    max(bitcast_int32(m) >> 23, 75) - 127   (the max() also maps the
    mels==0 -> eps=2^-52 case to -52 exactly).
"""

import os
import sys

for _p in ("/opt/trn_rl_repo",):
    if _p not in sys.path and os.path.isdir(_p):
        sys.path.append(_p)

import numpy as np
import ml_dtypes

import concourse.bass as bass
from concourse import bacc, mybir
from concourse.tile import TileContext
from concourse.bass_utils import run_bass_kernel_spmd

N_CORES = 8
B, T, FRAME = 4096, 32, 512
R = (B // N_CORES) * T  # 16384 rows of length 512 per core
N_MELS = 20
NQ = FRAME // 128  # 4 n-chunks of the transposed input
GR = 512  # rows per compute group (one PSUM bank of f32)

# DMA macro-blocks (rows): small first blocks so the pipeline fills quickly,
# and a smaller last block so the drain tail is short.
MACROS = [(0, 256), (256, 768), (1024, 1024)] + [
    (2048 + 2048 * i, 2048) for i in range(6)
] + [(14336, 1024), (15360, 1024)]
assert sum(rb for _, rb in MACROS) == R

f32 = mybir.dt.float32
f16 = mybir.dt.float16
bf16 = mybir.dt.bfloat16
i32 = mybir.dt.int32


def build_graph():
    """SPMD Bass graph for one core's shard.

    xt:  [NQ, 128, R] f16   transposed rows: xt[c, p, r] = x[perm(r), 128c+p]
    ce:  [2, 128, 128] f16  diag(hw_lo) @ cos matrix, even k (2,4,...,256)
    wo:  [NQ, 128, 128] f16 full windowed cos matrix, odd k (1,3,...,255)
    fbt: [2, 128, N_MELS] bf16  mel filterbank * 2^-75, split by k parity
         (the 2^-75 bias makes f32 subnormal flush implement the eps clamp:
          floor(log2(mels)) = (expbits(mels * 2^-75) >> 23) - 52, exact for
          mels > 2^-51, and the mels==0 -> eps path lands on -52 via the
          zero/subnormal exponent field)
    gr:  [128, 2] f32       window ratio hw_hi/hw_lo, n-chunked
    out: [R, N_MELS] f32
    """
    nc = bacc.Bacc(None, target_bir_lowering=False)
    xt_d = nc.declare_dram_parameter("xt", [NQ, 128, R], f16, isOutput=False)
    ce_d = nc.declare_dram_parameter("ce", [2, 128, 128], f16, isOutput=False)
    wo_d = nc.declare_dram_parameter("wo", [NQ, 128, 128], f16, isOutput=False)
    fbt_d = nc.declare_dram_parameter("fbt", [2, 128, N_MELS], bf16, isOutput=False)
    g_d = nc.declare_dram_parameter("gr", [128, 2], f32, isOutput=False)
    out_d = nc.declare_dram_parameter("out", [R, N_MELS], bf16, isOutput=True)

    with TileContext(nc) as tc:
        with (
            tc.tile_pool(name="consts", bufs=1) as consts,
            tc.tile_pool(name="xt", bufs=3) as xt_pool,
            tc.tile_pool(name="gx", bufs=3) as gx_pool,
            tc.tile_pool(name="u", bufs=3) as u_pool,
            tc.tile_pool(name="mag", bufs=3) as mag_pool,
            tc.tile_pool(name="fin", bufs=2) as fin_pool,
            tc.tile_pool(name="ps_y", bufs=3, space="PSUM") as ps_y_pool,
            tc.tile_pool(name="ps_m", bufs=2, space="PSUM") as ps_m_pool,
        ):
            # gr first (unblocks the folds), then matmul weights; macro-1's
            # input DMA follows right behind these on the sync queue.
            g_sb = consts.tile([128, 2], f32)
            nc.sync.dma_start(out=g_sb, in_=g_d[:, :])
            ce_sb = consts.tile([128, 2, 128], f16)
            nc.sync.dma_start(out=ce_sb, in_=ce_d.rearrange("c p k -> p c k"))
            wo_sb = consts.tile([128, NQ, 128], f16)
            nc.sync.dma_start(out=wo_sb, in_=wo_d.rearrange("c p k -> p c k"))
            fbt_sb = consts.tile([128, 2, N_MELS], bf16)
            nc.sync.dma_start(out=fbt_sb, in_=fbt_d.rearrange("e j m -> j e m"))

            def emit_load(m):
                """DMA + even-side window/fold for macro m."""
                r0, RB = MACROS[m]
                xt_sb = xt_pool.tile([128, NQ, RB], f16, name="xt_sb")
                q = nc.gpsimd if m % 2 == 0 else nc.sync
                q.dma_start(
                    out=xt_sb,
                    in_=xt_d[:, :, r0 : r0 + RB].rearrange("c p r -> p c r"),
                )
                # u[c] = x[c] + g[c]*x[c+2]  (hw_lo is folded into ce;
                # the odd side runs unfolded on the PE straight from xt)
                gx_sb = gx_pool.tile([128, 2, RB], f16, name="gx_sb")
                u_sb = u_pool.tile([128, 2, RB], f16, name="u_sb")
                for c in range(2):
                    nc.vector.tensor_scalar(
                        gx_sb[:, c], xt_sb[:, c + 2], g_sb[:, c : c + 1],
                        None, mybir.AluOpType.mult,
                    )
                    nc.vector.tensor_add(u_sb[:, c], xt_sb[:, c], gx_sb[:, c])
                return xt_sb, u_sb

            def emit_groups(m, xt_sb, u_sb):
                r0, RB = MACROS[m]
                S = RB // 128  # output slots per macro
                mels_ps = ps_m_pool.tile([128, S * N_MELS], f32, name="mels_ps")
                for off in range(0, RB, GR):
                    gr_n = min(GR, RB - off)
                    r = slice(off, off + gr_n)
                    # DFT: y[k, r] for even/odd k (f32 PSUM accumulate);
                    # odd first — it reads xt directly, no DVE dependency.
                    y_ps = ps_y_pool.tile([128, 2, gr_n], f32, name="y_ps")
                    for c in range(NQ):
                        nc.tensor.matmul(
                            y_ps[:, 1, :], wo_sb[:, c, :], xt_sb[:, c, r],
                            start=(c == 0), stop=(c == NQ - 1),
                        )
                    for c in range(2):
                        nc.tensor.matmul(
                            y_ps[:, 0, :], ce_sb[:, c, :], u_sb[:, c, r],
                            start=(c == 0), stop=(c == 1),
                        )
                    # mag = y^2 (fused PSUM -> SBUF bf16)
                    mag_sb = mag_pool.tile([128, 2, gr_n], bf16, name="mag_sb")
                    nc.scalar.activation(
                        mag_sb, y_ps, mybir.ActivationFunctionType.Square
                    )
                    # mel: mels[r, m] += mag[k, r].T @ fbt[k, m]
                    # (a whole macro's mels fit one PSUM bank)
                    for j in range(gr_n // 128):
                        jj = slice(j * 128, (j + 1) * 128)
                        s = off // 128 + j
                        for e in range(2):
                            nc.tensor.matmul(
                                mels_ps[:, s * N_MELS : (s + 1) * N_MELS],
                                mag_sb[:, e, jj], fbt_sb[:, e, :],
                                start=(e == 0), stop=(e == 1),
                            )
                # finalize: floor(log2(mels)) = expbits(mels * 2^-75) - 52
                e_sb = fin_pool.tile([128, S * N_MELS], i32, tag="e_sb",
                                     name="e_sb")
                nc.vector.tensor_scalar(
                    e_sb,
                    mels_ps.bitcast(i32),
                    23,
                    None,
                    mybir.AluOpType.logical_shift_right,
                )
                o_sb = fin_pool.tile([128, S * N_MELS], bf16, tag="o_sb",
                                     name="o_sb")
                nc.vector.tensor_scalar_sub(o_sb, e_sb, 52.0)
                # store: rows r0 + p*S + s are partition-contiguous in DRAM
                q = nc.sync if m % 2 == 0 else nc.gpsimd
                q.dma_start(
                    out=out_d[r0 : r0 + RB, :].rearrange(
                        "(p j) m -> p (j m)", j=S
                    ),
                    in_=o_sb,
                )

            # software pipeline: load macro m+1 (DMA + DVE folds) before
            # emitting macro m's matmul groups, so the DVE FIFO never parks
            # next macro's folds behind this macro's exponent shifts.
            pending = {0: emit_load(0)}
            for m in range(len(MACROS)):
                if m + 1 < len(MACROS):
                    pending[m + 1] = emit_load(m + 1)
                emit_groups(m, *pending.pop(m))
    nc.compile()
    return nc


def _prep_weights(filter_banks, hw):
    fb = np.asarray(filter_banks, dtype=np.float32)
    n_mels, n_bins = fb.shape  # (20, 257)
    assert n_mels == N_MELS and n_bins == FRAME // 2 + 1
    assert np.all(fb[:, 0] == 0.0), "parity-fold kernel needs an unused DC bin"

    k_even = np.arange(2, 257, 2)  # 128 bins: 2..256
    k_odd = np.arange(1, 256, 2)  # 128 bins: 1..255
    n256 = np.arange(256, dtype=np.float64)
    n512 = np.arange(512, dtype=np.float64)
    hw64 = np.asarray(hw, dtype=np.float64)
    ce = (hw64[:256, None]
          * np.cos(2.0 * np.pi * np.outer(n256, k_even) / FRAME) / FRAME)
    wo = (hw64[:, None]
          * np.cos(2.0 * np.pi * np.outer(n512, k_odd) / FRAME) / FRAME)
    ce = ce.reshape(2, 128, 128).astype(np.float16)
    wo = wo.reshape(NQ, 128, 128).astype(np.float16)

    # 2^-75 bias: the on-device eps clamp comes from subnormal flush of
    # mels * 2^-75 (see build_graph docstring). Exact power-of-2 scale.
    fbt = np.empty((2, 128, N_MELS), dtype=ml_dtypes.bfloat16)
    fbt[0] = (fb[:, k_even] * np.float32(2.0**-75)).T
    fbt[1] = (fb[:, k_odd] * np.float32(2.0**-75)).T

    g = (hw64[256:] / hw64[:256]).astype(np.float32)  # [256]
    gr = np.ascontiguousarray(g.reshape(2, 128).T)  # [128, 2]
    return ce, wo, fbt, gr


def _prep_inputs(x):
    """Shard, permute, transpose, cast: per core xt[c, p, r] with the macro-
    local row order r = 128*s + p_out chosen so stores are contiguous."""
    x16 = x.reshape(N_CORES, R, FRAME).astype(np.float16)
    parts = []
    for r0, RB in MACROS:
        S = RB // 128
        blk = x16[:, r0 : r0 + RB, :].reshape(N_CORES, 128, S, FRAME)
        # [core, p, s, n] -> [core, n, s, p] -> [core, NQ, 128, S*128]
        t = blk.transpose(0, 3, 2, 1).reshape(N_CORES, NQ, 128, RB)
        parts.append(t)
    xt = np.concatenate(parts, axis=3)  # [core, NQ, 128, R]
    return np.ascontiguousarray(xt)


_CACHE = {}


def _get_graph():
    if "nc" not in _CACHE:
        _CACHE["nc"] = build_graph()
    return _CACHE["nc"]


def kernel(inputs, filter_banks, hw, _trace=False):
    x = np.ascontiguousarray(np.asarray(inputs, dtype=np.float32))
    assert x.shape == (B, T, FRAME), x.shape
    ce, wo, fbt, gr = _prep_weights(filter_banks, hw)
    xt = _prep_inputs(x)

    nc = _get_graph()
    in_maps = [
        {"xt": xt[i], "ce": ce, "wo": wo, "fbt": fbt, "gr": gr}
        for i in range(N_CORES)
    ]
    res = run_bass_kernel_spmd(
        nc, in_maps, core_ids=list(range(N_CORES)), trace=_trace
    )
    out = np.stack(
        [np.asarray(res.results[i]["out"]) for i in range(N_CORES)], axis=0
    )
    # bf16 -> f32 is exact for these small-integer outputs
    out = out.astype(np.float32).reshape(B, T, N_MELS, 1)
    if _trace:
        kernel._last_result = res
    return out


# revision 30
# speedup vs baseline: 1.7402x; 1.0106x over previous
"""Trainium2 Bass kernel for AudioPreprocessingLayer.

Computes: floor(log2(mel_fb @ (rfft(x*hamming, norm=forward).real ** 2)))
for x of shape (4096, 32, 512), sharded batch-wise across 8 NeuronCores.

Key ideas:
  - rfft(.).real is a matmul with the cosine matrix C[n,k] = cos(2*pi*k*n/512)/512.
  - Parity fold: C[n+256, k] = (-1)^k C[n, k], so the even-k bins need only
    ue[n] = hw[n]x[n] + hw[n+256]x[n+256] and the odd-k bins only
    uo[n] = hw[n]x[n] - hw[n+256]x[n+256] — a 256-long contraction instead
    of 512: the DFT matmul work halves.
  - Window-in-weights: ue = hw_lo * (x_lo + g*x_hi) with g = hw_hi/hw_lo,
    and the outer hw_lo folds into the cosine weights. So the whole
    window+fold is 4 scalar_tensor_tensor ops per macro-block
    (out = (x_hi * ±g) + x_lo, g a per-partition scalar).
  - The host hands the kernel x already TRANSPOSED to [n, r] layout (a pure
    permutation, done during sharding), so no on-chip transpose is needed:
    the DMA-loaded tiles feed the DFT matmul directly with n on partitions.
  - The row order within each DMA macro-block is permuted host-side so the
    OUTPUT rows land partition-contiguous (big store descriptors).
  - fp16 end-to-end for x/u and the windowed cosine weights (better
    precision than a bf16 pipeline and full PE speed); mag/filterbank in
    bf16 (fp16 would flush y^2 subnormals); PSUM accumulation in f32.
  - floor(log2(m)) for positive fp32 m is exactly
    max(bitcast_int32(m) >> 23, 75) - 127   (the max() also maps the
    mels==0 -> eps=2^-52 case to -52 exactly).
"""

import os
import sys

for _p in ("/opt/trn_rl_repo",):
    if _p not in sys.path and os.path.isdir(_p):
        sys.path.append(_p)

import numpy as np
import ml_dtypes

import concourse.bass as bass
from concourse import bacc, mybir
from concourse.tile import TileContext
from concourse.bass_utils import run_bass_kernel_spmd

N_CORES = 8
B, T, FRAME = 4096, 32, 512
R = (B // N_CORES) * T  # 16384 rows of length 512 per core
N_MELS = 20
NQ = FRAME // 128  # 4 n-chunks of the transposed input
GR = 512  # rows per compute group (one PSUM bank of f32)
CHUNK_ORDER = [0, 2, 1, 3]  # n-chunk storage order: fold pairs adjacent

# DMA macro-blocks (rows): small first blocks so the pipeline fills quickly,
# and a smaller last block so the drain tail is short.
MACROS = [(0, 256), (256, 768), (1024, 1024)] + [
    (2048 + 2048 * i, 2048) for i in range(6)
] + [(14336, 1024), (15360, 1024)]
assert sum(rb for _, rb in MACROS) == R

f32 = mybir.dt.float32
f16 = mybir.dt.float16
bf16 = mybir.dt.bfloat16
i32 = mybir.dt.int32


def build_graph():
    """SPMD Bass graph for one core's shard.

    xt:  [NQ, 128, R] f16   transposed rows, n-chunks stored in order
         [0,2,1,3] so each half [0,2] / [1,3] is one contiguous DMA that
         feeds one fold: xt[i, p, r] = x[perm(r), 128*chunk(i)+p]
    ce:  [2, 128, 128] f16  diag(hw_lo) @ cos matrix, even k (2,4,...,256)
    wo:  [NQ, 128, 128] f16 full windowed cos matrix, odd k (1,3,...,255),
         n-chunks in the same [0,2,1,3] order as xt
    fbt: [2, 128, N_MELS] bf16  mel filterbank * 2^-75, split by k parity
         (the 2^-75 bias makes f32 subnormal flush implement the eps clamp:
          floor(log2(mels)) = (expbits(mels * 2^-75) >> 23) - 52, exact for
          mels > 2^-51, and the mels==0 -> eps path lands on -52 via the
          zero/subnormal exponent field)
    gr:  [128, 2] f32       window ratio hw_hi/hw_lo, n-chunked
    out: [R, N_MELS] f32
    """
    nc = bacc.Bacc(None, target_bir_lowering=False)
    xt_d = nc.declare_dram_parameter("xt", [NQ, 128, R], f16, isOutput=False)
    ce_d = nc.declare_dram_parameter("ce", [2, 128, 128], f16, isOutput=False)
    wo_d = nc.declare_dram_parameter("wo", [NQ, 128, 128], f16, isOutput=False)
    fbt_d = nc.declare_dram_parameter("fbt", [2, 128, N_MELS], bf16, isOutput=False)
    g_d = nc.declare_dram_parameter("gr", [128, 2], f32, isOutput=False)
    out_d = nc.declare_dram_parameter("out", [R, N_MELS], bf16, isOutput=True)

    with TileContext(nc) as tc:
        with (
            tc.tile_pool(name="consts", bufs=1) as consts,
            tc.tile_pool(name="xta", bufs=4) as xta_pool,
            tc.tile_pool(name="xtb", bufs=4) as xtb_pool,
            tc.tile_pool(name="gx", bufs=3) as gx_pool,
            tc.tile_pool(name="u", bufs=3) as u_pool,
            tc.tile_pool(name="mag", bufs=3) as mag_pool,
            tc.tile_pool(name="fin", bufs=2) as fin_pool,
            tc.tile_pool(name="ps_y", bufs=3, space="PSUM") as ps_y_pool,
            tc.tile_pool(name="ps_m", bufs=2, space="PSUM") as ps_m_pool,
        ):
            # gr first (unblocks the folds), then matmul weights; macro-1's
            # input DMA follows right behind these on the sync queue.
            g_sb = consts.tile([128, 2], f32)
            nc.sync.dma_start(out=g_sb, in_=g_d[:, :])
            ce_sb = consts.tile([128, 2, 128], f16)
            nc.sync.dma_start(out=ce_sb, in_=ce_d.rearrange("c p k -> p c k"))
            wo_sb = consts.tile([128, NQ, 128], f16)
            nc.sync.dma_start(out=wo_sb, in_=wo_d.rearrange("c p k -> p c k"))
            fbt_sb = consts.tile([128, 2, N_MELS], bf16)
            nc.sync.dma_start(out=fbt_sb, in_=fbt_d.rearrange("e j m -> j e m"))

            def emit_load(m):
                """Two half-DMAs + even-side window/fold for macro m. Each
                half carries the (x_lo, x_hi) pair one fold needs, so the
                folds start after half the macro's data has landed."""
                r0, RB = MACROS[m]
                xta_sb = xta_pool.tile([128, 2, RB], f16, name="xta_sb")
                nc.gpsimd.dma_start(
                    out=xta_sb,
                    in_=xt_d[0:2, :, r0 : r0 + RB].rearrange("c p r -> p c r"),
                )
                xtb_sb = xtb_pool.tile([128, 2, RB], f16, name="xtb_sb")
                nc.sync.dma_start(
                    out=xtb_sb,
                    in_=xt_d[2:4, :, r0 : r0 + RB].rearrange("c p r -> p c r"),
                )
                # u[c] = x[c] + g[c]*x[c+2]  (hw_lo is folded into ce;
                # the odd side runs unfolded on the PE straight from xt)
                gx_sb = gx_pool.tile([128, 2, RB], f16, name="gx_sb")
                u_sb = u_pool.tile([128, 2, RB], f16, name="u_sb")
                for c, h_sb in ((0, xta_sb), (1, xtb_sb)):
                    nc.vector.tensor_scalar(
                        gx_sb[:, c], h_sb[:, 1], g_sb[:, c : c + 1],
                        None, mybir.AluOpType.mult,
                    )
                    nc.vector.tensor_add(u_sb[:, c], h_sb[:, 0], gx_sb[:, c])
                return (xta_sb, xtb_sb), u_sb

            def emit_groups(m, xt_sb, u_sb):
                xta_sb, xtb_sb = xt_sb
                r0, RB = MACROS[m]
                S = RB // 128  # output slots per macro
                mels_ps = ps_m_pool.tile([128, S * N_MELS], f32, name="mels_ps")
                for off in range(0, RB, GR):
                    gr_n = min(GR, RB - off)
                    r = slice(off, off + gr_n)
                    # DFT: y[k, r] for even/odd k (f32 PSUM accumulate);
                    # odd first — it reads xt directly, no DVE dependency.
                    y_ps = ps_y_pool.tile([128, 2, gr_n], f32, name="y_ps")
                    odd_srcs = (xta_sb[:, 0, r], xta_sb[:, 1, r],
                                xtb_sb[:, 0, r], xtb_sb[:, 1, r])
                    for c in range(NQ):
                        nc.tensor.matmul(
                            y_ps[:, 1, :], wo_sb[:, c, :], odd_srcs[c],
                            start=(c == 0), stop=(c == NQ - 1),
                        )
                    for c in range(2):
                        nc.tensor.matmul(
                            y_ps[:, 0, :], ce_sb[:, c, :], u_sb[:, c, r],
                            start=(c == 0), stop=(c == 1),
                        )
                    # mag = y^2 (fused PSUM -> SBUF bf16)
                    mag_sb = mag_pool.tile([128, 2, gr_n], bf16, name="mag_sb")
                    nc.scalar.activation(
                        mag_sb, y_ps, mybir.ActivationFunctionType.Square
                    )
                    # mel: mels[r, m] += mag[k, r].T @ fbt[k, m]
                    # (a whole macro's mels fit one PSUM bank)
                    for j in range(gr_n // 128):
                        jj = slice(j * 128, (j + 1) * 128)
                        s = off // 128 + j
                        for e in range(2):
                            nc.tensor.matmul(
                                mels_ps[:, s * N_MELS : (s + 1) * N_MELS],
                                mag_sb[:, e, jj], fbt_sb[:, e, :],
                                start=(e == 0), stop=(e == 1),
                            )
                # finalize: floor(log2(mels)) = expbits(mels * 2^-75) - 52
                e_sb = fin_pool.tile([128, S * N_MELS], i32, tag="e_sb",
                                     name="e_sb")
                nc.vector.tensor_scalar(
                    e_sb,
                    mels_ps.bitcast(i32),
                    23,
                    None,
                    mybir.AluOpType.logical_shift_right,
                )
                o_sb = fin_pool.tile([128, S * N_MELS], bf16, tag="o_sb",
                                     name="o_sb")
                nc.vector.tensor_scalar_sub(o_sb, e_sb, 52.0)
                # store: rows r0 + p*S + s are partition-contiguous in DRAM
                q = nc.sync if m % 2 == 0 else nc.gpsimd
                q.dma_start(
                    out=out_d[r0 : r0 + RB, :].rearrange(
                        "(p j) m -> p (j m)", j=S
                    ),
                    in_=o_sb,
                )

            # software pipeline: load macro m+1 (DMA + DVE folds) before
            # emitting macro m's matmul groups, so the DVE FIFO never parks
            # next macro's folds behind this macro's exponent shifts.
            pending = {0: emit_load(0)}
            for m in range(len(MACROS)):
                if m + 1 < len(MACROS):
                    pending[m + 1] = emit_load(m + 1)
                emit_groups(m, *pending.pop(m))
    nc.compile()
    return nc


def _prep_weights(filter_banks, hw):
    fb = np.asarray(filter_banks, dtype=np.float32)
    n_mels, n_bins = fb.shape  # (20, 257)
    assert n_mels == N_MELS and n_bins == FRAME // 2 + 1
    assert np.all(fb[:, 0] == 0.0), "parity-fold kernel needs an unused DC bin"

    k_even = np.arange(2, 257, 2)  # 128 bins: 2..256
    k_odd = np.arange(1, 256, 2)  # 128 bins: 1..255
    n256 = np.arange(256, dtype=np.float64)
    n512 = np.arange(512, dtype=np.float64)
    hw64 = np.asarray(hw, dtype=np.float64)
    ce = (hw64[:256, None]
          * np.cos(2.0 * np.pi * np.outer(n256, k_even) / FRAME) / FRAME)
    wo = (hw64[:, None]
          * np.cos(2.0 * np.pi * np.outer(n512, k_odd) / FRAME) / FRAME)
    ce = ce.reshape(2, 128, 128).astype(np.float16)
    wo = wo.reshape(NQ, 128, 128)[CHUNK_ORDER]
    wo = np.ascontiguousarray(wo).astype(np.float16)

    # 2^-75 bias: the on-device eps clamp comes from subnormal flush of
    # mels * 2^-75 (see build_graph docstring). Exact power-of-2 scale.
    fbt = np.empty((2, 128, N_MELS), dtype=ml_dtypes.bfloat16)
    fbt[0] = (fb[:, k_even] * np.float32(2.0**-75)).T
    fbt[1] = (fb[:, k_odd] * np.float32(2.0**-75)).T

    g = (hw64[256:] / hw64[:256]).astype(np.float32)  # [256]
    gr = np.ascontiguousarray(g.reshape(2, 128).T)  # [128, 2]
    return ce, wo, fbt, gr


def _prep_inputs(x):
    """Shard, permute, transpose, cast: per core xt[c, p, r] with the macro-
    local row order r = 128*s + p_out chosen so stores are contiguous."""
    x16 = x.reshape(N_CORES, R, FRAME).astype(np.float16)
    parts = []
    for r0, RB in MACROS:
        S = RB // 128
        blk = x16[:, r0 : r0 + RB, :].reshape(N_CORES, 128, S, FRAME)
        # [core, p, s, n] -> [core, n, s, p] -> [core, NQ, 128, S*128]
        t = blk.transpose(0, 3, 2, 1).reshape(N_CORES, NQ, 128, RB)
        parts.append(t[:, CHUNK_ORDER])
    xt = np.concatenate(parts, axis=3)  # [core, NQ, 128, R]
    return np.ascontiguousarray(xt)


_CACHE = {}


def _get_graph():
    if "nc" not in _CACHE:
        _CACHE["nc"] = build_graph()
    return _CACHE["nc"]


def kernel(inputs, filter_banks, hw, _trace=False):
    x = np.ascontiguousarray(np.asarray(inputs, dtype=np.float32))
    assert x.shape == (B, T, FRAME), x.shape
    ce, wo, fbt, gr = _prep_weights(filter_banks, hw)
    xt = _prep_inputs(x)

    nc = _get_graph()
    in_maps = [
        {"xt": xt[i], "ce": ce, "wo": wo, "fbt": fbt, "gr": gr}
        for i in range(N_CORES)
    ]
    res = run_bass_kernel_spmd(
        nc, in_maps, core_ids=list(range(N_CORES)), trace=_trace
    )
    out = np.stack(
        [np.asarray(res.results[i]["out"]) for i in range(N_CORES)], axis=0
    )
    # bf16 -> f32 is exact for these small-integer outputs
    out = out.astype(np.float32).reshape(B, T, N_MELS, 1)
    if _trace:
        kernel._last_result = res
    return out
